# revision 1
# baseline (speedup 1.0000x reference)
"""GraphSAGE-mean GNN (3 layers + classifier) on 8 Trainium2 NeuronCores.

Strategy (data-parallel over nodes, sharded by dst):
  - Nodes padded 50000 -> 50176, degree-sorted and dealt round-robin over the
    8 cores so every core sees a near-identical degree profile (SPMD: one
    compiled program for all cores). Per core: 6272 nodes = 49 tiles of 128.
  - Neighbor gather uses the custom Q7 dma_gather instruction (int16 indices,
    4 SWDGE queues). Since indices are signed int16 (< 32768), the gather
    table is addressed through two windows: A = rows [0, 32767] (sources on
    cores 0-4) and B = rows [17409, 50175] (sources on cores 5-7). Zero-slot
    padding points at dummy-node rows which are explicitly zeroed.
  - Gathered neighbor blocks [128 nodes x 128 feat] are accumulated on the
    tensor engine (identity matmuls into PSUM) to keep the vector engine off
    the SBUF port that SWDGE descriptor generation needs.
  - deg_inv scaling + PSUM->SBUF copies run on the scalar (ACT) engine.
  - Dense layer matmuls run feature-major: out = lhsT.T @ rhs with the weight
    as lhsT and h^T as rhs. Aggregates are transposed per-tile via PE.
  - Node features for the next layer's gather are re-transposed to node-major
    fp16, DMA'd to DRAM and AllGathered across the 8 cores.
"""

import os
import numpy as np

P = 128
NC = 8
N = 50000
E = 800000
NPAD = 50176
SH = 6272  # nodes per core
TILES = 49
H = 128
NLAYERS = 3
ODIM = 40
ACUT = 31360  # sources with new id < ACUT use window A (cores 0-4)
B0 = 17409  # window B base row; idx = id - B0 (<= 32766 for id < 50176)
AZERO = ACUT - 1  # core 4's last node: a dummy (zeroed) row, < 32768
BZERO = NPAD - 1 - B0  # core 7's last node as B-window index
NQ = int(os.environ.get('GNN_NQ', '4'))
GCHUNK = int(os.environ.get('GNN_GCHUNK', '16'))  # max 128-blocks per gather call
GBUFS = int(os.environ.get('GNN_GBUFS', '5'))
DENSE_BLOCKS = [(i * 512, 512) for i in range(12)] + [(6144, 128)]
N_DUMMY = SH - N // NC  # 22 dummy nodes per core, at the tail of tile 48

last_results = None


def _prep(edge_index):
    """Host-side graph preprocessing: permutation, per-tile neighbor slots."""
    src = edge_index[0].astype(np.int64)
    dst = edge_index[1].astype(np.int64)
    deg = np.bincount(dst, minlength=N).astype(np.float32)
    dinv = 1.0 / np.maximum(deg, 1.0)

    # pass 1: global degree sort, deal round-robin -> core assignment
    order = np.argsort(-deg, kind="stable")  # rank -> orig node
    core_of = np.empty(N, np.int64)
    core_of[order] = np.arange(N) % NC

    # pass 2: within-core order by (degA, degB) desc; degA counts in-edges
    # whose source core is in 0..4 (window A). Two re-deal rounds align the
    # per-core (degA, degB) profiles, shrinking the cross-core tile maxima.
    for _ in range(2):
        srcA = core_of[src] < 5
        degA = np.bincount(dst[srcA], minlength=N).astype(np.int64)
        degB = np.bincount(dst[~srcA], minlength=N).astype(np.int64)
        key = (degA // 10) * 1000000 + (degB // 4) * 1000 \
            + degA % 10 * 10 + degB % 4
        order = np.argsort(-key, kind="stable")
        core_of = np.empty(N, np.int64)
        core_of[order] = np.arange(N) % NC
    srcA = core_of[src] < 5
    degA = np.bincount(dst[srcA], minlength=N).astype(np.int64)
    degB = np.bincount(dst[~srcA], minlength=N).astype(np.int64)

    newid = np.empty(N, np.int64)
    for c in range(NC):
        nodes = np.flatnonzero(core_of == c)
        key = (degA[nodes] // 10) * 1000000 + (degB[nodes] // 4) * 1000 \
            + degA[nodes] % 10 * 10 + degB[nodes] % 4
        nodes = nodes[np.argsort(-key, kind="stable")]
        newid[nodes] = c * SH + np.arange(len(nodes))  # dummies occupy the tail

    ns = newid[src]
    nd = newid[dst]
    isA = ns < ACUT

    # per-(core, tile) block counts; uniform across cores for SPMD
    def build(mask, zero_idx, rebase):
        ns_h, nd_h = ns[mask], nd[mask]
        o = np.argsort(nd_h, kind="stable")
        ns_h, nd_h = ns_h[o], nd_h[o]
        # cumcount within each dst node
        n_edges = len(nd_h)
        if n_edges == 0:
            return np.zeros(NC * NPAD // P // NC, np.int64), None, None
        firsts = np.r_[0, np.flatnonzero(np.diff(nd_h)) + 1]
        grp = np.zeros(n_edges, np.int64)
        grp[firsts] = 1
        grp = np.cumsum(grp) - 1
        d_in_node = np.arange(n_edges) - firsts[grp]
        cnt = np.bincount(nd_h, minlength=NC * SH)  # per new-node count
        cnt_t = cnt.reshape(NC, TILES, P)
        D_t = cnt_t.max(axis=2).max(axis=0)  # [TILES], max over cores
        base = np.r_[0, np.cumsum(D_t * P)]  # slot base per tile
        CA = int(base[-1])  # slots per core
        arr16 = np.full((NC, 16, CA // 16), zero_idx, np.int64)
        c_h = nd_h // SH
        pos = nd_h % SH
        t_h = pos // P
        p_h = pos % P
        flat = base[t_h] + d_in_node * P + p_h
        val = ns_h - rebase
        arr16[c_h, flat % 16, flat // 16] = val
        return D_t, base, arr16

    DA, baseA, arrA = build(isA, AZERO, 0)
    DB, baseB, arrB = build(~isA, BZERO, B0)

    inv = np.empty(N, np.int64)  # only defined for real nodes
    inv = newid[np.arange(N)]  # orig -> new
    return dict(
        dinv=dinv, newid=inv, DA=DA, DB=DB, baseA=baseA, baseB=baseB,
        arrA=arrA, arrB=arrB,
    )



def _patch_swdge_lane_by_queue():
    """Bind DMASW sem lanes to SWDGE queues so multi-queue SWDGE keeps the
    per-queue semaphore invariant the ucode reclaim needs. Lane 2q serves
    queue q's normal (gen_mode=0) DMAs; lane 2q+1 its PREPARE_ONLY preps —
    deterministic, so trace-time sem= choices match pass-1 lane assignment.
    Returns an undo function."""
    import concourse.tile_sem_assignment as tsa
    import concourse.mybir as mybir
    from concourse import bass_isa
    from concourse.tile_scheduler import DMAInst

    orig = tsa.TileClockTick._assign_tick

    def patched(self, inst):
        try:
            is_pool_dma = (
                isinstance(inst, DMAInst)
                and inst.engine == mybir.EngineType.Pool
                and not isinstance(inst, bass_isa.UserSyncedRemoteDMADescs)
            )
        except Exception:
            is_pool_dma = False
        if is_pool_dma:
            q = int(getattr(inst, "queue_num", 0) or 0)
            counters = getattr(self, "_queue_lane_ctr", None)
            if counters is None:
                counters = {}
                self._queue_lane_ctr = counters
            k = counters.get(q, 0)
            counters[q] = k + 1
            lanes_per_q = max(1, self.swdge_sem_count // NQ)
            self.next_sw_dma_idx = (q * lanes_per_q + k % lanes_per_q) % (
                self.swdge_sem_count
            )
        return orig(self, inst)

    tsa.TileClockTick._assign_tick = patched

    def undo():
        tsa.TileClockTick._assign_tick = orig

    return undo


def _build_program(DA, DB, baseA, baseB, CA, CB):
    import concourse.bass as bass
    import concourse.bacc as bacc
    import concourse.tile as tile
    import concourse.mybir as mybir

    f32 = mybir.dt.float32
    f16 = mybir.dt.float16
    i16 = mybir.dt.int16
    AF = mybir.ActivationFunctionType

    nc = bacc.Bacc("TRN2", target_bir_lowering=False, debug=False,
                   num_devices=NC, num_swdge_queues=NQ)

    # ---- I/O ----
    xT = nc.dram_tensor("xT", [P, SH], f32, kind="ExternalInput")
    idxA = nc.dram_tensor("idxA", [P, CA // 16], i16, kind="ExternalInput")
    idxB = nc.dram_tensor("idxB", [P, CB // 16], i16, kind="ExternalInput")
    dinv_in = nc.dram_tensor("dinv", [P, TILES], f32, kind="ExternalInput")
    w_in = nc.dram_tensor("w_in", [H, H], f32, kind="ExternalInput")
    b_in = nc.dram_tensor("b_in", [H, 1], f32, kind="ExternalInput")
    wl_in = nc.dram_tensor("wl", [H, NLAYERS * H], f32, kind="ExternalInput")
    wr_in = nc.dram_tensor("wr", [H, NLAYERS * H], f16, kind="ExternalInput")
    bl_in = nc.dram_tensor("bl", [H, NLAYERS], f32, kind="ExternalInput")
    wc1_in = nc.dram_tensor("wc1", [H, H], f32, kind="ExternalInput")
    bc1_in = nc.dram_tensor("bc1", [H, 1], f32, kind="ExternalInput")
    wc2_in = nc.dram_tensor("wc2", [H, ODIM], f32, kind="ExternalInput")
    bc2_in = nc.dram_tensor("bc2", [ODIM, 1], f32, kind="ExternalInput")
    ident_in = nc.dram_tensor("ident", [P, P], f16, kind="ExternalInput")
    dmask_in = nc.dram_tensor("dmask", [P, 1], f32, kind="ExternalInput")
    outT = nc.dram_tensor("outT", [ODIM, SH], f32, kind="ExternalOutput")

    with tile.TileContext(nc) as tc:
        with (
            tc.tile_pool(name="consts", bufs=1) as consts,
            tc.tile_pool(name="big", bufs=1) as big,
            tc.tile_pool(name="hT", bufs=2) as hTp,
            tc.tile_pool(name="aggTp", bufs=1) as aggTp,
            tc.tile_pool(name="hnm", bufs=1) as hnmp,
            tc.tile_pool(name="gA", bufs=GBUFS) as gAp,
            tc.tile_pool(name="gB", bufs=GBUFS) as gBp,
            tc.tile_pool(name="small", bufs=4) as small,
            tc.tile_pool(name="psagg", bufs=4, space="PSUM") as psagg,
            tc.tile_pool(name="pstp", bufs=2, space="PSUM") as pstp,
            tc.tile_pool(name="psz", bufs=1, space="PSUM") as psz,
            tc.tile_pool(name="dram", bufs=1, space="DRAM") as dram,
        ):
            # ---- load constants ----
            idxA_sb = consts.tile([P, CA // 16], i16)
            nc.sync.dma_start(out=idxA_sb[:], in_=idxA[:])
            idxB_sb = consts.tile([P, CB // 16], i16)
            nc.sync.dma_start(out=idxB_sb[:], in_=idxB[:])
            dinv_sb = consts.tile([P, TILES], f32)
            nc.sync.dma_start(out=dinv_sb[:], in_=dinv_in[:])
            w_in_sb = consts.tile([H, H], f32)
            nc.sync.dma_start(out=w_in_sb[:], in_=w_in[:])
            b_in_sb = consts.tile([H, 1], f32)
            nc.sync.dma_start(out=b_in_sb[:], in_=b_in[:])
            wl_sb = consts.tile([H, NLAYERS * H], f32)
            nc.sync.dma_start(out=wl_sb[:], in_=wl_in[:])
            wr_sb = consts.tile([H, NLAYERS * H], f16)
            nc.sync.dma_start(out=wr_sb[:], in_=wr_in[:])
            bl_sb = consts.tile([H, NLAYERS], f32)
            nc.sync.dma_start(out=bl_sb[:], in_=bl_in[:])
            wc1_sb = consts.tile([H, H], f32)
            nc.sync.dma_start(out=wc1_sb[:], in_=wc1_in[:])
            bc1_sb = consts.tile([H, 1], f32)
            nc.sync.dma_start(out=bc1_sb[:], in_=bc1_in[:])
            wc2_sb = consts.tile([H, ODIM], f32)
            nc.sync.dma_start(out=wc2_sb[:], in_=wc2_in[:])
            bc2_sb = consts.tile([ODIM, 1], f32)
            nc.sync.dma_start(out=bc2_sb[:], in_=bc2_in[:])
            ident_sb = consts.tile([P, P], f16)
            nc.sync.dma_start(out=ident_sb[:], in_=ident_in[:])
            dmask_sb = consts.tile([P, 1], f32)
            nc.sync.dma_start(out=dmask_sb[:], in_=dmask_in[:])
            xsum = big.tile([P, SH], f32)
            nc.vector.memset(xsum[:], 0.0)

            tabs = [dram.tile([NPAD, H], f16, addr_space="Shared",
                               name=f"tab{i}") for i in range(NLAYERS)]
            ag_ins = [dram.tile([SH, H], f16, name=f"ag_in{i}")
                      for i in range(NLAYERS)]

            swsems = tc.sems.swdge_block()

            # static per-layer gather job list (identical every layer):
            # (tile, window, nblocks, block_offset) in emission order
            def _layer_jobs():
                jobs = []
                for gi in ([len(DENSE_BLOCKS) - 1]
                           + list(range(len(DENSE_BLOCKS) - 1))):
                    off0, w0 = DENSE_BLOCKS[gi]
                    for t in range(off0 // P, off0 // P + w0 // P):
                        for d, which in ((int(DA[t]), 0), (int(DB[t]), 1)):
                            if not d:
                                continue
                            nch = -(-d // GCHUNK)
                            o2 = 0
                            for i in range(nch):
                                s = d // nch + (1 if i < d % nch else 0)
                                jobs.append((t, which, s, o2))
                                o2 += s
                return jobs

            JOBS = _layer_jobs()
            qb = [0] * NQ
            QOF = []  # greedy drain-balanced queue per job
            for (_, _, s, _) in JOBS:
                q = min(range(NQ), key=lambda i: qb[i])
                qb[q] += s
                QOF.append(q)
            # first job index of each tile, to locate a tile's jobs quickly
            TILE_JOBS = {}
            for j, (t, _, _, _) in enumerate(JOBS):
                TILE_JOBS.setdefault(t, []).append(j)
            NPREF = int(os.environ.get("GNN_NPREF", "0"))

            def emit_job(j, tab, gtiles, prep):
                t, which, s, off = JOBS[j]
                pool, tag = (gAp, "gA") if which == 0 else (gBp, "gB")
                base, idx_sb = (baseA, idxA_sb) if which == 0 else (baseB,
                                                                   idxB_sb)
                tab_ap = tab[:] if which == 0 else tab[B0:, :]
                g = pool.tile([P, s * P], f16, tag=tag, name=tag)
                col0 = base[t] // 16 + off * 8
                kw = {}
                if prep:
                    kw = dict(prepare_only=True,
                              sem=swsems[2 * QOF[j] + 1])
                nc.gpsimd.dma_gather(
                    g[:].rearrange("p (b e) -> p b e", e=H),
                    tab_ap,
                    idx_sb[:, col0 : col0 + s * 8],
                    s * P, s * P, H,
                    single_packet=False, queue_num=QOF[j], **kw)
                gtiles[j] = g

            def emit_prefix(tab, gtiles):
                """pre-generate descriptors for a prefix of the job list; the
                DMAs fire via trigger_dma once the AllGather lands. Capped at
                GBUFS jobs per pool so no prep reuses an untriggered prep's
                ring slot (that WAR would deadlock the engine)."""
                per_pool = {0: 0, 1: 0}
                done = 0
                for j in range(min(2 * NPREF, len(JOBS))):
                    which = JOBS[j][1]
                    if per_pool[which] >= GBUFS:
                        continue
                    per_pool[which] += 1
                    emit_job(j, tab, gtiles, prep=True)
                    done += 1
                    if done >= NPREF:
                        break

            def fire_prefix_triggers(gtiles, tab):
                """fire the prefix preps once tab's AllGather landed.

                Tile does not thread the collective->table dependency onto
                trigger_dma itself, so emit a tiny Pool-engine read of tab
                first: Tile gives IT the collective wait, and the engine's
                in-order dispatch holds the triggers behind it."""
                if not gtiles:
                    return
                probe = small.tile([1, H], f16, tag="tabprobe",
                                   name="tabprobe")
                nc.gpsimd.dma_start(out=probe[:], in_=tab[0:1, :])
                qs = {QOF[j] for j in gtiles}
                for q in sorted(qs):
                    nc.gpsimd.trigger_dma(count=None, queue_num=q)

            def gather_tile(t, tab, aggT, gtiles):
                """segment-sum one 128-dst tile into aggT cols; issues the
                tile's gathers unless they were pre-prepped."""
                da, db = int(DA[t]), int(DB[t])
                ps = psagg.tile([P, P], f32, tag="agg", name="psa")
                nblk = da + db
                k = 0
                for j in TILE_JOBS[t]:
                    if j not in gtiles:
                        emit_job(j, tab, gtiles, prep=False)
                    g = gtiles[j]
                    s = JOBS[j][2]
                    for dd in range(s):
                        nc.tensor.matmul(
                            out=ps[:], lhsT=ident_sb[:],
                            rhs=g[:, dd * P : (dd + 1) * P],
                            start=(k == 0), stop=(k == nblk - 1))
                        k += 1
                # deg_inv scale + cast to fp16 on the idle DVE engine so the
                # ACT queue never gates the psagg release chain
                agg_nm = small.tile([P, P], f16, tag="aggnm", name="aggnm")
                nc.vector.tensor_scalar_mul(
                    out=agg_nm[:], in0=ps[:], scalar1=dinv_sb[:, t : t + 1])
                pst = pstp.tile([P, P], f16, tag="tp", name="pst2")
                nc.tensor.transpose(out=pst[:], in_=agg_nm[:],
                                    identity=ident_sb[:])
                nc.vector.tensor_scalar_mul(
                    out=aggT[:, t * P : (t + 1) * P], in0=pst[:], scalar1=1.0)

            def pack_tiles(hT_src, hnm, t0, t1):
                """fp32->fp16 + transpose tiles [t0,t1) of hT_src into hnm."""
                for t in range(t0, t1):
                    blk16 = small.tile([P, P], f16, tag="blk16", name="blk16")
                    nc.scalar.activation(
                        out=blk16[:], in_=hT_src[:, t * P : (t + 1) * P],
                        func=AF.Copy)
                    pst = pstp.tile([P, P], f16, tag="tp", name="pst")
                    nc.tensor.transpose(out=pst[:], in_=blk16[:],
                                        identity=ident_sb[:])
                    if t == TILES - 1:
                        # zero the dummy-node rows (tail partitions) via mask
                        nc.scalar.activation(
                            out=hnm[:, t * P : (t + 1) * P], in_=pst[:],
                            func=AF.Copy, scale=dmask_sb[:, 0:1])
                    else:
                        nc.scalar.activation(
                            out=hnm[:, t * P : (t + 1) * P], in_=pst[:],
                            func=AF.Copy)

            def fire_allgather(hnm, tab, ag_in):
                nc.sync.dma_start(
                    out=ag_in[:].rearrange("(t p) f -> p t f", p=P),
                    in_=hnm[:].rearrange("p (t f) -> p t f", f=P))
                if not os.environ.get("GNN_SKIP_AG"):
                    nc.gpsimd.collective_compute(
                        "AllGather", mybir.AluOpType.bypass,
                        replica_groups=[list(range(NC))],
                        ins=[ag_in[:]], outs=[tab[:]])

            # tile-groups matching the 512-col dense blocks; a small group
            # first primes the gather->matmul pipeline after each AllGather
            GROUPS = [(off // P, w // P, off, w) for off, w in DENSE_BLOCKS]
            GORDER = [len(GROUPS) - 1] + list(range(len(GROUPS) - 1))

            # ---- layer 0: h0 = x @ W_in + b_in (pack interleaved) ----
            hT = hTp.tile([P, SH], f32, tag="hT", name="hT0")
            hnm = hnmp.tile([P, TILES * P], f16, tag="hnm", name="hnm0")
            for ci in range(0, len(GROUPS), 3):
                grp = GROUPS[ci : ci + 3]
                c0 = grp[0][2]
                cw = sum(g[3] for g in grp)
                xblk = small.tile([P, 3 * 512], f32, tag="xblk", name="xblk",
                                  bufs=2)
                nc.sync.dma_start(out=xblk[:, :cw], in_=xT[:, c0 : c0 + cw])
                for toff, tw, off, w in grp:
                    ps = psz.tile([P, 512], f32, tag="z", name="ps0")
                    nc.tensor.matmul(out=ps[:, :w], lhsT=w_in_sb[:],
                                     rhs=xblk[:, off - c0 : off - c0 + w],
                                     start=True, stop=True)
                    nc.scalar.activation(out=hT[:, off : off + w],
                                         in_=ps[:, :w],
                                         func=AF.Identity,
                                         bias=b_in_sb[:, 0:1])
                    pack_tiles(hT, hnm, toff, toff + tw)
            gtiles = {}
            emit_prefix(tabs[0], gtiles)
            fire_allgather(hnm, tabs[0], ag_ins[0])
            fire_prefix_triggers(gtiles, tabs[0])

            # ---- GNN layers: gather/dense/pack/classifier interleaved ----
            outT_sb = big.tile([ODIM, SH], f32)
            for layer in range(NLAYERS):
                last = layer == NLAYERS - 1
                aggT = aggTp.tile([P, SH], f16, tag="aggT", name="aggT")
                hT2 = hTp.tile([P, SH], f32, tag="hT", name=f"hT{layer + 1}")
                if not last:
                    hnm = hnmp.tile([P, TILES * P], f16, tag="hnm",
                                    name=f"hnm{layer + 1}")
                for gi in GORDER:
                    toff, tw, off, w = GROUPS[gi]
                    for t in range(toff, toff + tw):
                        gather_tile(t, tabs[layer], aggT, gtiles)
                    ps = psz.tile([P, 512], f32, tag="z", name=f"psz{layer}")
                    nc.tensor.matmul(
                        out=ps[:, :w], lhsT=wl_sb[:, layer * H : (layer + 1) * H],
                        rhs=hT[:, off : off + w], start=True, stop=False)
                    nc.tensor.matmul(
                        out=ps[:, :w], lhsT=wr_sb[:, layer * H : (layer + 1) * H],
                        rhs=aggT[:, off : off + w], start=False, stop=True,
                        skip_group_check=True)
                    nc.scalar.activation(
                        out=hT2[:, off : off + w], in_=ps[:, :w],
                        func=AF.Relu, bias=bl_sb[:, layer : layer + 1])
                    nc.vector.tensor_add(
                        out=xsum[:, off : off + w], in0=xsum[:, off : off + w],
                        in1=hT2[:, off : off + w])
                    if not last:
                        pack_tiles(hT2, hnm, toff, toff + tw)
                    else:
                        # classifier on the finished xsum columns
                        psc = psz.tile([P, 512], f32, tag="z", name="psc1")
                        nc.tensor.matmul(out=psc[:, :w], lhsT=wc1_sb[:],
                                         rhs=xsum[:, off : off + w],
                                         start=True, stop=True)
                        hc = small.tile([P, 512], f32, tag="hc", name="hc",
                                        bufs=2)
                        nc.scalar.activation(out=hc[:, :w], in_=psc[:, :w],
                                             func=AF.Relu, bias=bc1_sb[:, 0:1])
                        ps2 = psz.tile([ODIM, 512], f32, tag="z2", name="psc2",
                                       bufs=1)
                        nc.tensor.matmul(out=ps2[:, :w], lhsT=wc2_sb[:],
                                         rhs=hc[:, :w], start=True, stop=True)
                        nc.scalar.activation(out=outT_sb[:, off : off + w],
                                             in_=ps2[:, :w], func=AF.Identity,
                                             bias=bc2_sb[:, 0:1])
                if not last:
                    gtiles = {}
                    emit_prefix(tabs[layer + 1], gtiles)
                    fire_allgather(hnm, tabs[layer + 1], ag_ins[layer + 1])
                    fire_prefix_triggers(gtiles, tabs[layer + 1])
                hT = hT2
            nc.sync.dma_start(out=outT[:], in_=outT_sb[:])

    nc.compile()
    return nc


def kernel(x, edge_index, W_in, b_in, Wl, Wr, bl, Wc1, bc1, Wc2, bc2):
    global last_results
    from concourse.bass_utils import run_bass_kernel_spmd

    x = np.asarray(x, np.float32)
    edge_index = np.asarray(edge_index)
    meta = _prep(edge_index)
    DA, DB = meta["DA"], meta["DB"]
    baseA, baseB = meta["baseA"], meta["baseB"]
    CA, CB = int(baseA[-1]), int(baseB[-1])
    newid = meta["newid"]

    if NQ > 1:
        undo = _patch_swdge_lane_by_queue()
        try:
            nc = _build_program(DA, DB, baseA, baseB, CA, CB)
        finally:
            undo()
    else:
        nc = _build_program(DA, DB, baseA, baseB, CA, CB)

    # ---- per-core inputs ----
    dinv_full = np.ones(NC * SH, np.float32)
    dinv_full[newid] = meta["dinv"]
    x_full = np.zeros((NC * SH, H), np.float32)
    x_full[newid] = x

    ident = np.eye(P, dtype=np.float16)
    dmask = np.ones((P, 1), np.float32)
    dmask[P - N_DUMMY :] = 0.0
    wl_cat = np.concatenate([Wl[i] for i in range(NLAYERS)], 1).astype(np.float32)
    wr_cat = np.concatenate([Wr[i] for i in range(NLAYERS)], 1).astype(np.float16)
    bl_T = np.asarray(bl, np.float32).T.copy()  # [H, 3]

    in_maps = []
    for c in range(NC):
        sl = slice(c * SH, (c + 1) * SH)
        in_maps.append({
            "xT": x_full[sl].T.copy(),
            "idxA": np.tile(meta["arrA"][c], (8, 1)).astype(np.int16),
            "idxB": np.tile(meta["arrB"][c], (8, 1)).astype(np.int16),
            "dinv": dinv_full[sl].reshape(TILES, P).T.copy(),
            "w_in": np.asarray(W_in, np.float32),
            "b_in": np.asarray(b_in, np.float32).reshape(H, 1),
            "wl": wl_cat, "wr": wr_cat, "bl": bl_T,
            "wc1": np.asarray(Wc1, np.float32),
            "bc1": np.asarray(bc1, np.float32).reshape(H, 1),
            "wc2": np.asarray(Wc2, np.float32),
            "bc2": np.asarray(bc2, np.float32).reshape(ODIM, 1),
            "ident": ident,
            "dmask": dmask,
        })

    trace = bool(int(os.environ.get("GNN_TRACE", "0")))
    res = run_bass_kernel_spmd(nc, in_maps, list(range(NC)), trace=trace)
    last_results = res

    big = np.concatenate([res.results[c]["outT"] for c in range(NC)], axis=1)
    out = big.T[newid]  # [N, ODIM]
    return out.astype(np.float32)



# revision 17
# speedup vs baseline: 1.0599x; 1.0599x over previous
"""GraphSAGE-mean GNN (3 layers + classifier) on 8 Trainium2 NeuronCores.

Strategy (data-parallel over dst nodes, SPMD):
  - Nodes padded 50000 -> 50176, degree-sorted and dealt round-robin over
    the 8 cores. Per core: 6272 nodes = 49 tiles of 128.
  - Neighbor gather via SWDGE dma_gather (node-major 256B rows, int16
    indices, 4 queues round-robin). The gather table is split into two
    HALVES by source tile range, each with its own AllGather + table:
      p1 = every core's tiles 0-23  (rows c*3200 + i,        i < 3072)
      p2 = every core's tiles 24-48 (rows c*3200 + i - 3072, i >= 3072)
    Both windows stay under the int16 limit (< 25600 rows), and the p1
    AllGather fires mid-layer (right after tile 23's features are
    packed), so the next layer's p1 gathers overlap the p2 AllGather.
  - Gathered [128 x 128] blocks are segment-summed on the tensor engine
    (identity matmuls into PSUM); deg_inv scale + fp16 cast on DVE; agg
    transposed per-tile on PE into feature-major for the dense matmuls.
  - Dense layers run fully in fp16 (weights, h, classifier).
  - Gather jobs are uniform 16-block (2048-slot) chunks crossing tile
    boundaries, queues strictly round-robin; the first FRONT p1 chunks
    of each layer are emitted before any p2 chunk so the in-order Pool
    engine never parks on the p2 AllGather while p1 work is available.
"""

import os
import numpy as np

P = 128
NC = 8
N = 50000
E = 800000
NPAD = 50176
SH = 6272  # nodes per core
TILES = 49
H = 128
NLAYERS = 3
ODIM = 40
T1 = 24  # tiles per core in window p1
R1 = T1 * P  # 3072 rows per core in p1
RS = 3200  # row stride per core in both windows (p1 pads 3072->3200)
Z1 = R1  # p1 zero row: core 0's pad block
Z2 = NC * RS - 1  # p2 zero row: core 7's dummy tail node
N_DUMMY = SH - N // NC  # 22 dummy nodes per core, at the tail of tile 48
NQ = int(os.environ.get('GNN_NQ', '4'))
GCHUNK = int(os.environ.get('GNN_GCHUNK', '16'))  # 128-blocks per gather call
GBUFS = int(os.environ.get('GNN_GBUFS', '8'))
FRONT = int(os.environ.get('GNN_FRONT', '8'))  # p1 chunks emitted up front
DENSE_BLOCKS = [(i * 512, 512) for i in range(12)] + [(6144, 128)]
AG1_GROUP = 5  # fire p1 AllGather after this dense group's pack

last_results = None


def _prep(edge_index):
    """Host-side graph preprocessing: permutation, per-tile neighbor slots."""
    src = edge_index[0].astype(np.int64)
    dst = edge_index[1].astype(np.int64)
    deg = np.bincount(dst, minlength=N).astype(np.float32)
    dinv = 1.0 / np.maximum(deg, 1.0)

    # window membership is FIXED up front: p1 sources = the NC*R1 nodes
    # with highest in-degree. This keeps (deg1, deg2) stable so a single
    # aligned deal minimizes the per-tile padding maxima.
    degi = deg.astype(np.int64)
    order_deg = np.argsort(-degi, kind="stable")
    is1n = np.zeros(N, bool)
    is1n[order_deg[:NC * R1]] = True
    deg1 = np.bincount(dst[is1n[src]], minlength=N)
    deg2 = np.bincount(dst[~is1n[src]], minlength=N)
    key = (is1n.astype(np.int64) * 10**12 + (deg1 // 8) * 10**6
           + (deg2 // 4) * 10**3 + (deg1 % 8) * 10 + deg2 % 4)
    order = np.argsort(-key, kind="stable")
    core_of = np.empty(N, np.int64)
    core_of[order] = np.arange(N) % NC
    rank = np.empty(N, np.int64)
    for c in range(NC):
        nodes = np.flatnonzero(core_of == c)
        nodes = nodes[np.argsort(-key[nodes], kind="stable")]
        rank[nodes] = np.arange(len(nodes))

    newid = core_of * SH + rank  # dummies implicitly occupy the tail ranks

    ns = newid[src]
    nd = newid[dst]
    is1 = is1n[src]

    def build(mask, zero_idx, val_fn):
        ns_h, nd_h = ns[mask], nd[mask]
        o = np.argsort(nd_h, kind="stable")
        ns_h, nd_h = ns_h[o], nd_h[o]
        n_edges = len(nd_h)
        firsts = np.r_[0, np.flatnonzero(np.diff(nd_h)) + 1]
        grp = np.zeros(n_edges, np.int64)
        grp[firsts] = 1
        grp = np.cumsum(grp) - 1
        d_in_node = np.arange(n_edges) - firsts[grp]
        cnt = np.bincount(nd_h, minlength=NC * SH)
        cnt_t = cnt.reshape(NC, TILES, P)
        D_t = cnt_t.max(axis=2).max(axis=0)  # [TILES], max over cores
        D_t = np.maximum(D_t, 1)
        base = np.r_[0, np.cumsum(D_t * P)]  # slot base per tile
        C = int(base[-1])  # slots per core
        arr16 = np.full((NC, 16, C // 16), zero_idx, np.int64)
        c_h = nd_h // SH
        pos = nd_h % SH
        t_h = pos // P
        p_h = pos % P
        flat = base[t_h] + d_in_node * P + p_h
        arr16[c_h, flat % 16, flat // 16] = val_fn(ns_h)
        return D_t, base, arr16

    D1, base1, arr1 = build(
        is1, Z1, lambda s: (s // SH) * RS + (s % SH))
    D2, base2, arr2 = build(
        ~is1, Z2, lambda s: (s // SH) * RS + (s % SH) - R1)

    return dict(
        dinv=dinv, newid=newid, D1=D1, D2=D2, base1=base1, base2=base2,
        arr1=arr1, arr2=arr2,
    )


def _patch_swdge_lane_by_queue():
    """Bind DMASW sem lanes to SWDGE queues so multi-queue SWDGE keeps the
    per-queue semaphore invariant the ucode reclaim needs."""
    import concourse.tile_sem_assignment as tsa
    import concourse.mybir as mybir
    from concourse import bass_isa
    from concourse.tile_scheduler import DMAInst

    orig = tsa.TileClockTick._assign_tick

    def patched(self, inst):
        try:
            is_pool_dma = (
                isinstance(inst, DMAInst)
                and inst.engine == mybir.EngineType.Pool
                and not isinstance(inst, bass_isa.UserSyncedRemoteDMADescs)
            )
        except Exception:
            is_pool_dma = False
        if is_pool_dma:
            q = int(getattr(inst, "queue_num", 0) or 0)
            counters = getattr(self, "_queue_lane_ctr", None)
            if counters is None:
                counters = {}
                self._queue_lane_ctr = counters
            k = counters.get(q, 0)
            counters[q] = k + 1
            lanes_per_q = max(1, self.swdge_sem_count // NQ)
            self.next_sw_dma_idx = (q * lanes_per_q + k % lanes_per_q) % (
                self.swdge_sem_count
            )
        return orig(self, inst)

    tsa.TileClockTick._assign_tick = patched

    def undo():
        tsa.TileClockTick._assign_tick = orig

    return undo


def _build_program(D1, D2, base1, base2, C1, C2):
    import concourse.bass as bass
    import concourse.bacc as bacc
    import concourse.tile as tile
    import concourse.mybir as mybir
    import bisect

    f32 = mybir.dt.float32
    f16 = mybir.dt.float16
    i16 = mybir.dt.int16
    AF = mybir.ActivationFunctionType

    nc = bacc.Bacc("TRN2", target_bir_lowering=False, debug=False,
                   num_devices=NC, num_swdge_queues=NQ)

    # ---- I/O ----
    xT = nc.dram_tensor("xT", [P, SH], f16, kind="ExternalInput")
    idx1 = nc.dram_tensor("idx1", [P, C1 // 16], i16, kind="ExternalInput")
    idx2 = nc.dram_tensor("idx2", [P, C2 // 16], i16, kind="ExternalInput")
    dinv_in = nc.dram_tensor("dinv", [P, TILES], f32, kind="ExternalInput")
    w_in = nc.dram_tensor("w_in", [H, H], f16, kind="ExternalInput")
    b_in = nc.dram_tensor("b_in", [H, 1], f32, kind="ExternalInput")
    wl_in = nc.dram_tensor("wl", [H, NLAYERS * H], f16, kind="ExternalInput")
    wr_in = nc.dram_tensor("wr", [H, NLAYERS * H], f16, kind="ExternalInput")
    bl_in = nc.dram_tensor("bl", [H, NLAYERS], f32, kind="ExternalInput")
    wc1_in = nc.dram_tensor("wc1", [H, H], f16, kind="ExternalInput")
    bc1_in = nc.dram_tensor("bc1", [H, 1], f32, kind="ExternalInput")
    wc2_in = nc.dram_tensor("wc2", [H, ODIM], f16, kind="ExternalInput")
    bc2_in = nc.dram_tensor("bc2", [ODIM, 1], f32, kind="ExternalInput")
    ident_in = nc.dram_tensor("ident", [P, P], f16, kind="ExternalInput")
    dmask_in = nc.dram_tensor("dmask", [P, 1], f32, kind="ExternalInput")
    outT = nc.dram_tensor("outT", [ODIM, SH], f32, kind="ExternalOutput")

    with tile.TileContext(nc) as tc:
        with (
            tc.tile_pool(name="consts", bufs=1) as consts,
            tc.tile_pool(name="big", bufs=1) as big,
            tc.tile_pool(name="hT", bufs=2) as hTp,
            tc.tile_pool(name="aggTp", bufs=1) as aggTp,
            tc.tile_pool(name="g1", bufs=GBUFS) as g1p,
            tc.tile_pool(name="g2", bufs=GBUFS) as g2p,
            tc.tile_pool(name="small", bufs=4) as small,
            tc.tile_pool(name="psagg", bufs=4, space="PSUM") as psagg,
            tc.tile_pool(name="pstp", bufs=2, space="PSUM") as pstp,
            tc.tile_pool(name="psz", bufs=1, space="PSUM") as psz,
            tc.tile_pool(name="dram", bufs=1, space="DRAM") as dram,
        ):
            # ---- load constants ----
            idx1_sb = consts.tile([P, C1 // 16], i16)
            nc.sync.dma_start(out=idx1_sb[:], in_=idx1[:])
            idx2_sb = consts.tile([P, C2 // 16], i16)
            nc.sync.dma_start(out=idx2_sb[:], in_=idx2[:])
            dinv_sb = consts.tile([P, TILES], f32)
            nc.sync.dma_start(out=dinv_sb[:], in_=dinv_in[:])
            w_in_sb = consts.tile([H, H], f16)
            nc.sync.dma_start(out=w_in_sb[:], in_=w_in[:])
            b_in_sb = consts.tile([H, 1], f32)
            nc.sync.dma_start(out=b_in_sb[:], in_=b_in[:])
            wl_sb = consts.tile([H, NLAYERS * H], f16)
            nc.sync.dma_start(out=wl_sb[:], in_=wl_in[:])
            wr_sb = consts.tile([H, NLAYERS * H], f16)
            nc.sync.dma_start(out=wr_sb[:], in_=wr_in[:])
            bl_sb = consts.tile([H, NLAYERS], f32)
            nc.sync.dma_start(out=bl_sb[:], in_=bl_in[:])
            wc1_sb = consts.tile([H, H], f16)
            nc.sync.dma_start(out=wc1_sb[:], in_=wc1_in[:])
            bc1_sb = consts.tile([H, 1], f32)
            nc.sync.dma_start(out=bc1_sb[:], in_=bc1_in[:])
            wc2_sb = consts.tile([H, ODIM], f16)
            nc.sync.dma_start(out=wc2_sb[:], in_=wc2_in[:])
            bc2_sb = consts.tile([ODIM, 1], f32)
            nc.sync.dma_start(out=bc2_sb[:], in_=bc2_in[:])
            ident_sb = consts.tile([P, P], f16)
            nc.sync.dma_start(out=ident_sb[:], in_=ident_in[:])
            dmask_sb = consts.tile([P, 1], f32)
            nc.sync.dma_start(out=dmask_sb[:], in_=dmask_in[:])
            zero_sb = consts.tile([P, P], f16)
            nc.vector.memset(zero_sb[:], 0.0)
            xsum = big.tile([P, SH], f16)
            nc.vector.memset(xsum[:], 0.0)

            tab1s = [dram.tile([NC * RS, H], f16, addr_space="Shared",
                               name=f"tab1_{i}") for i in range(NLAYERS)]
            tab2s = [dram.tile([NC * RS, H], f16, addr_space="Shared",
                               name=f"tab2_{i}") for i in range(NLAYERS)]
            ag1_ins = [dram.tile([RS, H], f16, name=f"ag1_in{i}")
                       for i in range(NLAYERS)]
            ag2_ins = [dram.tile([RS, H], f16, name=f"ag2_in{i}")
                       for i in range(NLAYERS)]

            # ---- gather job list: uniform GCHUNK-block chunks over each
            # window's flat block stream; p1 frontloaded, then interleaved
            B1, B2 = C1 // P, C2 // P
            tb1 = [int(b) // P for b in base1]  # tile boundaries in blocks
            tb2 = [int(b) // P for b in base2]
            raw = []  # (which, b0, s)
            for which, nb in ((0, B1), (1, B2)):
                for b0 in range(0, nb, GCHUNK):
                    raw.append((which, b0, min(GCHUNK, nb - b0)))

            def job_tile(job):
                which, b0, s = job
                tb = tb1 if which == 0 else tb2
                return bisect.bisect_right(tb, b0) - 1

            p1jobs = sorted([j for j in raw if j[0] == 0], key=lambda j: j[1])
            p2jobs = sorted([j for j in raw if j[0] == 1], key=lambda j: j[1])
            body = sorted(p1jobs[FRONT:] + p2jobs,
                          key=lambda j: (job_tile(j), j[0], j[1]))
            JOBS = p1jobs[:FRONT] + body
            JOB_OF = {}
            for pos, (which, b0, s) in enumerate(JOBS):
                JOB_OF[(which, b0 // GCHUNK)] = pos
            QOF = [j % NQ for j in range(len(JOBS))]  # strict round-robin

            def emit_job(j, tabs, gtiles):
                which, b0, s = JOBS[j]
                pool, tag = (g1p, "g1") if which == 0 else (g2p, "g2")
                idx_sb = idx1_sb if which == 0 else idx2_sb
                g = pool.tile([P, s * P], f16, tag=tag, name=tag)
                nc.gpsimd.dma_gather(
                    g[:].rearrange("p (b e) -> p b e", e=H),
                    tabs[which][:],
                    idx_sb[:, b0 * 8: b0 * 8 + s * 8],
                    s * P, s * P, H,
                    single_packet=False, queue_num=QOF[j])
                gtiles[j] = g

            def gather_tile(t, tabs, aggT, gtiles):
                """segment-sum one 128-dst tile into aggT cols."""
                d1, d2 = int(D1[t]), int(D2[t])
                ps = psagg.tile([P, P], f32, tag="agg", name="psa")
                nblk = d1 + d2
                k = 0
                for which, tb in ((0, tb1), (1, tb2)):
                    for b in range(tb[t], tb[t + 1]):
                        j = JOB_OF[(which, b // GCHUNK)]
                        if j not in gtiles:
                            emit_job(j, tabs, gtiles)
                        g = gtiles[j]
                        off = b - JOBS[j][1]
                        nc.tensor.matmul(
                            out=ps[:], lhsT=ident_sb[:],
                            rhs=g[:, off * P: (off + 1) * P],
                            start=(k == 0), stop=(k == nblk - 1))
                        k += 1
                agg_nm = small.tile([P, P], f16, tag="aggnm", name="aggnm")
                nc.vector.tensor_scalar_mul(
                    out=agg_nm[:], in0=ps[:], scalar1=dinv_sb[:, t: t + 1])
                pst = pstp.tile([P, P], f16, tag="tp", name="pst2")
                nc.tensor.transpose(out=pst[:], in_=agg_nm[:],
                                    identity=ident_sb[:])
                nc.vector.tensor_scalar_mul(
                    out=aggT[:, t * P: (t + 1) * P], in0=pst[:], scalar1=1.0)

            def pack_tiles(hT_src, ag1, ag2, t0, t1):
                """transpose tiles [t0,t1) of hT_src (fp16) to node-major
                and DMA the rows into the proper AllGather input."""
                for t in range(t0, t1):
                    pst = pstp.tile([P, P], f16, tag="tp", name="pst")
                    nc.tensor.transpose(
                        out=pst[:], in_=hT_src[:, t * P: (t + 1) * P],
                        identity=ident_sb[:])
                    ev = small.tile([P, P], f16, tag="ev", name="ev", bufs=3)
                    if t == TILES - 1:
                        nc.scalar.activation(out=ev[:], in_=pst[:],
                                             func=AF.Copy,
                                             scale=dmask_sb[:, 0:1])
                    else:
                        nc.scalar.activation(out=ev[:], in_=pst[:],
                                             func=AF.Copy)
                    if t < T1:
                        nc.sync.dma_start(
                            out=ag1[t * P: (t + 1) * P, :], in_=ev[:])
                    else:
                        t2 = t - T1
                        nc.sync.dma_start(
                            out=ag2[t2 * P: (t2 + 1) * P, :], in_=ev[:])

            def fire_ag(tab, ag_in):
                if not os.environ.get("GNN_SKIP_AG"):
                    nc.gpsimd.collective_compute(
                        "AllGather", mybir.AluOpType.bypass,
                        replica_groups=[list(range(NC))],
                        ins=[ag_in[:]], outs=[tab[:]])

            GROUPS = [(off // P, w // P, off, w) for off, w in DENSE_BLOCKS]

            # ---- layer 0: h0 = x @ W_in + b_in ----
            hT = hTp.tile([P, SH], f16, tag="hT", name="hT0")
            # zero-pad block of ag1 (rows R1..RS) for p1 padding slots
            nc.sync.dma_start(out=ag1_ins[0][R1:RS, :], in_=zero_sb[:])
            for gi, (toff, tw, off, w) in enumerate(GROUPS):
                xblk = small.tile([P, 512], f16, tag="xblk", name="xblk",
                                  bufs=2)
                nc.sync.dma_start(out=xblk[:, :w], in_=xT[:, off: off + w])
                ps = psz.tile([P, 512], f32, tag="z", name="ps0")
                nc.tensor.matmul(out=ps[:, :w], lhsT=w_in_sb[:],
                                 rhs=xblk[:, :w], start=True, stop=True)
                nc.scalar.activation(out=hT[:, off: off + w],
                                     in_=ps[:, :w], func=AF.Identity,
                                     bias=b_in_sb[:, 0:1])
                pack_tiles(hT, ag1_ins[0], ag2_ins[0], toff, toff + tw)
                if gi == AG1_GROUP:
                    fire_ag(tab1s[0], ag1_ins[0])
            fire_ag(tab2s[0], ag2_ins[0])

            # ---- GNN layers ----
            outT_sb = big.tile([ODIM, SH], f32)
            for layer in range(NLAYERS):
                last = layer == NLAYERS - 1
                aggT = aggTp.tile([P, SH], f16, tag="aggT", name="aggT")
                hT2 = hTp.tile([P, SH], f16, tag="hT", name=f"hT{layer + 1}")
                tabs = (tab1s[layer], tab2s[layer])
                gtiles = {}
                # frontloaded p1 chunks (their table landed mid-previous
                # layer) keep the queues busy during the p2 AllGather
                for j in range(min(FRONT, len(JOBS))):
                    if JOBS[j][0] == 0:
                        emit_job(j, tabs, gtiles)
                if not last:
                    nc.sync.dma_start(out=ag1_ins[layer + 1][R1:RS, :],
                                      in_=zero_sb[:])
                for gi, (toff, tw, off, w) in enumerate(GROUPS):
                    for t in range(toff, toff + tw):
                        gather_tile(t, tabs, aggT, gtiles)
                    ps = psz.tile([P, 512], f32, tag="z", name=f"psz{layer}")
                    nc.tensor.matmul(
                        out=ps[:, :w],
                        lhsT=wl_sb[:, layer * H: (layer + 1) * H],
                        rhs=hT[:, off: off + w], start=True, stop=False)
                    nc.tensor.matmul(
                        out=ps[:, :w],
                        lhsT=wr_sb[:, layer * H: (layer + 1) * H],
                        rhs=aggT[:, off: off + w], start=False, stop=True,
                        skip_group_check=True)
                    nc.scalar.activation(
                        out=hT2[:, off: off + w], in_=ps[:, :w],
                        func=AF.Relu, bias=bl_sb[:, layer: layer + 1])
                    nc.vector.tensor_add(
                        out=xsum[:, off: off + w],
                        in0=xsum[:, off: off + w],
                        in1=hT2[:, off: off + w])
                    if not last:
                        pack_tiles(hT2, ag1_ins[layer + 1],
                                   ag2_ins[layer + 1], toff, toff + tw)
                        if gi == AG1_GROUP:
                            fire_ag(tab1s[layer + 1], ag1_ins[layer + 1])
                    else:
                        psc = psz.tile([P, 512], f32, tag="z", name="psc1")
                        nc.tensor.matmul(out=psc[:, :w], lhsT=wc1_sb[:],
                                         rhs=xsum[:, off: off + w],
                                         start=True, stop=True)
                        hc = small.tile([P, 512], f16, tag="hc", name="hc",
                                        bufs=2)
                        nc.scalar.activation(out=hc[:, :w], in_=psc[:, :w],
                                             func=AF.Relu,
                                             bias=bc1_sb[:, 0:1])
                        ps2 = psz.tile([ODIM, 512], f32, tag="z2",
                                       name="psc2", bufs=1)
                        nc.tensor.matmul(out=ps2[:, :w], lhsT=wc2_sb[:],
                                         rhs=hc[:, :w], start=True,
                                         stop=True)
                        nc.scalar.activation(out=outT_sb[:, off: off + w],
                                             in_=ps2[:, :w],
                                             func=AF.Identity,
                                             bias=bc2_sb[:, 0:1])
                if not last:
                    fire_ag(tab2s[layer + 1], ag2_ins[layer + 1])
                hT = hT2
            nc.sync.dma_start(out=outT[:], in_=outT_sb[:])

    nc.compile()
    return nc


def kernel(x, edge_index, W_in, b_in, Wl, Wr, bl, Wc1, bc1, Wc2, bc2):
    global last_results
    from concourse.bass_utils import run_bass_kernel_spmd

    x = np.asarray(x, np.float32)
    edge_index = np.asarray(edge_index)
    meta = _prep(edge_index)
    D1, D2 = meta["D1"], meta["D2"]
    base1, base2 = meta["base1"], meta["base2"]
    C1, C2 = int(base1[-1]), int(base2[-1])
    newid = meta["newid"]

    if NQ > 1:
        undo = _patch_swdge_lane_by_queue()
        try:
            nc = _build_program(D1, D2, base1, base2, C1, C2)
        finally:
            undo()
    else:
        nc = _build_program(D1, D2, base1, base2, C1, C2)

    # ---- per-core inputs ----
    dinv_full = np.ones(NC * SH, np.float32)
    dinv_full[newid] = meta["dinv"]
    x_full = np.zeros((NC * SH, H), np.float32)
    x_full[newid] = x

    ident = np.eye(P, dtype=np.float16)
    dmask = np.ones((P, 1), np.float32)
    dmask[P - N_DUMMY:] = 0.0
    wl_cat = np.concatenate([Wl[i] for i in range(NLAYERS)], 1).astype(
        np.float16)
    wr_cat = np.concatenate([Wr[i] for i in range(NLAYERS)], 1).astype(
        np.float16)
    bl_T = np.asarray(bl, np.float32).T.copy()  # [H, 3]

    in_maps = []
    for c in range(NC):
        sl = slice(c * SH, (c + 1) * SH)
        in_maps.append({
            "xT": x_full[sl].T.astype(np.float16).copy(),
            "idx1": np.tile(meta["arr1"][c], (8, 1)).astype(np.int16),
            "idx2": np.tile(meta["arr2"][c], (8, 1)).astype(np.int16),
            "dinv": dinv_full[sl].reshape(TILES, P).T.copy(),
            "w_in": np.asarray(W_in, np.float16),
            "b_in": np.asarray(b_in, np.float32).reshape(H, 1),
            "wl": wl_cat, "wr": wr_cat, "bl": bl_T,
            "wc1": np.asarray(Wc1, np.float16),
            "bc1": np.asarray(bc1, np.float32).reshape(H, 1),
            "wc2": np.asarray(Wc2, np.float16),
            "bc2": np.asarray(bc2, np.float32).reshape(ODIM, 1),
            "ident": ident,
            "dmask": dmask,
        })

    trace = bool(int(os.environ.get("GNN_TRACE", "0")))
    res = run_bass_kernel_spmd(nc, in_maps, list(range(NC)), trace=trace)
    last_results = res

    big_out = np.concatenate([res.results[c]["outT"] for c in range(NC)],
                             axis=1)
    out = big_out.T[newid]  # [N, ODIM]
    return out.astype(np.float32)


# revision 18
# speedup vs baseline: 1.1076x; 1.0450x over previous
"""GraphSAGE-mean GNN (3 layers + classifier) on 8 Trainium2 NeuronCores.

Strategy (data-parallel over nodes, sharded by dst):
  - Nodes padded 50000 -> 50176, degree-sorted and dealt round-robin over the
    8 cores so every core sees a near-identical degree profile (SPMD: one
    compiled program for all cores). Per core: 6272 nodes = 49 tiles of 128.
  - Neighbor gather uses the custom Q7 dma_gather instruction (int16 indices,
    4 SWDGE queues). Since indices are signed int16 (< 32768), the gather
    table is addressed through two windows: A = rows [0, 32767] (sources on
    cores 0-4) and B = rows [17409, 50175] (sources on cores 5-7). Zero-slot
    padding points at dummy-node rows which are explicitly zeroed.
  - Gathered neighbor blocks [128 nodes x 128 feat] are accumulated on the
    tensor engine (identity matmuls into PSUM) to keep the vector engine off
    the SBUF port that SWDGE descriptor generation needs.
  - deg_inv scaling + PSUM->SBUF copies run on the scalar (ACT) engine.
  - Dense layer matmuls run feature-major: out = lhsT.T @ rhs with the weight
    as lhsT and h^T as rhs. Aggregates are transposed per-tile via PE.
  - Node features for the next layer's gather are re-transposed to node-major
    fp16, DMA'd to DRAM and AllGathered across the 8 cores.
"""

import os
import numpy as np

P = 128
NC = 8
N = 50000
E = 800000
NPAD = 50176
SH = 6272  # nodes per core
TILES = 49
H = 128
NLAYERS = 3
ODIM = 40
ACUT = 31360  # sources with new id < ACUT use window A (cores 0-4)
B0 = 17409  # window B base row; idx = id - B0 (<= 32766 for id < 50176)
AZERO = ACUT - 1  # core 4's last node: a dummy (zeroed) row, < 32768
BZERO = NPAD - 1 - B0  # core 7's last node as B-window index
NQ = int(os.environ.get('GNN_NQ', '4'))
GCHUNK = int(os.environ.get('GNN_GCHUNK', '16'))  # 128-blocks per gather call
GBUFS = int(os.environ.get('GNN_GBUFS', '5'))
DENSE_BLOCKS = [(i * 512, 512) for i in range(12)] + [(6144, 128)]
N_DUMMY = SH - N // NC  # 22 dummy nodes per core, at the tail of tile 48

last_results = None


def _prep(edge_index):
    """Host-side graph preprocessing: permutation, per-tile neighbor slots."""
    src = edge_index[0].astype(np.int64)
    dst = edge_index[1].astype(np.int64)
    deg = np.bincount(dst, minlength=N).astype(np.float32)
    dinv = 1.0 / np.maximum(deg, 1.0)

    # pass 1: global degree sort, deal round-robin -> core assignment
    order = np.argsort(-deg, kind="stable")  # rank -> orig node
    core_of = np.empty(N, np.int64)
    core_of[order] = np.arange(N) % NC

    # pass 2: within-core order by (degA, degB) desc; degA counts in-edges
    # whose source core is in 0..4 (window A). Two re-deal rounds align the
    # per-core (degA, degB) profiles, shrinking the cross-core tile maxima.
    for _ in range(2):
        srcA = core_of[src] < 5
        degA = np.bincount(dst[srcA], minlength=N).astype(np.int64)
        degB = np.bincount(dst[~srcA], minlength=N).astype(np.int64)
        key = (degA // 10) * 1000000 + (degB // 4) * 1000 \
            + degA % 10 * 10 + degB % 4
        order = np.argsort(-key, kind="stable")
        core_of = np.empty(N, np.int64)
        core_of[order] = np.arange(N) % NC
    srcA = core_of[src] < 5
    degA = np.bincount(dst[srcA], minlength=N).astype(np.int64)
    degB = np.bincount(dst[~srcA], minlength=N).astype(np.int64)

    newid = np.empty(N, np.int64)
    for c in range(NC):
        nodes = np.flatnonzero(core_of == c)
        key = (degA[nodes] // 10) * 1000000 + (degB[nodes] // 4) * 1000 \
            + degA[nodes] % 10 * 10 + degB[nodes] % 4
        nodes = nodes[np.argsort(-key, kind="stable")]
        newid[nodes] = c * SH + np.arange(len(nodes))  # dummies occupy the tail

    ns = newid[src]
    nd = newid[dst]
    isA = ns < ACUT

    # per-(core, tile) block counts; uniform across cores for SPMD
    def build(mask, zero_idx, rebase):
        ns_h, nd_h = ns[mask], nd[mask]
        o = np.argsort(nd_h, kind="stable")
        ns_h, nd_h = ns_h[o], nd_h[o]
        # cumcount within each dst node
        n_edges = len(nd_h)
        if n_edges == 0:
            return np.zeros(NC * NPAD // P // NC, np.int64), None, None
        firsts = np.r_[0, np.flatnonzero(np.diff(nd_h)) + 1]
        grp = np.zeros(n_edges, np.int64)
        grp[firsts] = 1
        grp = np.cumsum(grp) - 1
        d_in_node = np.arange(n_edges) - firsts[grp]
        cnt = np.bincount(nd_h, minlength=NC * SH)  # per new-node count
        cnt_t = cnt.reshape(NC, TILES, P)
        D_t = cnt_t.max(axis=2).max(axis=0)  # [TILES], max over cores
        base = np.r_[0, np.cumsum(D_t * P)]  # slot base per tile
        CA = int(base[-1])  # slots per core
        arr16 = np.full((NC, 16, CA // 16), zero_idx, np.int64)
        c_h = nd_h // SH
        pos = nd_h % SH
        t_h = pos // P
        p_h = pos % P
        flat = base[t_h] + d_in_node * P + p_h
        val = ns_h - rebase
        arr16[c_h, flat % 16, flat // 16] = val
        return D_t, base, arr16

    DA, baseA, arrA = build(isA, AZERO, 0)
    DB, baseB, arrB = build(~isA, BZERO, B0)

    inv = np.empty(N, np.int64)  # only defined for real nodes
    inv = newid[np.arange(N)]  # orig -> new
    return dict(
        dinv=dinv, newid=inv, DA=DA, DB=DB, baseA=baseA, baseB=baseB,
        arrA=arrA, arrB=arrB,
    )



def _patch_swdge_lane_by_queue():
    """Bind DMASW sem lanes to SWDGE queues so multi-queue SWDGE keeps the
    per-queue semaphore invariant the ucode reclaim needs. Lane 2q serves
    queue q's normal (gen_mode=0) DMAs; lane 2q+1 its PREPARE_ONLY preps —
    deterministic, so trace-time sem= choices match pass-1 lane assignment.
    Returns an undo function."""
    import concourse.tile_sem_assignment as tsa
    import concourse.mybir as mybir
    from concourse import bass_isa
    from concourse.tile_scheduler import DMAInst

    orig = tsa.TileClockTick._assign_tick

    def patched(self, inst):
        try:
            is_pool_dma = (
                isinstance(inst, DMAInst)
                and inst.engine == mybir.EngineType.Pool
                and not isinstance(inst, bass_isa.UserSyncedRemoteDMADescs)
            )
        except Exception:
            is_pool_dma = False
        if is_pool_dma:
            q = int(getattr(inst, "queue_num", 0) or 0)
            counters = getattr(self, "_queue_lane_ctr", None)
            if counters is None:
                counters = {}
                self._queue_lane_ctr = counters
            k = counters.get(q, 0)
            counters[q] = k + 1
            lanes_per_q = max(1, self.swdge_sem_count // NQ)
            self.next_sw_dma_idx = (q * lanes_per_q + k % lanes_per_q) % (
                self.swdge_sem_count
            )
        return orig(self, inst)

    tsa.TileClockTick._assign_tick = patched

    def undo():
        tsa.TileClockTick._assign_tick = orig

    return undo


def _build_program(DA, DB, baseA, baseB, CA, CB):
    import concourse.bass as bass
    import concourse.bacc as bacc
    import concourse.tile as tile
    import concourse.mybir as mybir

    f32 = mybir.dt.float32
    f16 = mybir.dt.float16
    i16 = mybir.dt.int16
    AF = mybir.ActivationFunctionType

    nc = bacc.Bacc("TRN2", target_bir_lowering=False, debug=False,
                   num_devices=NC, num_swdge_queues=NQ)

    # ---- I/O ----
    xT = nc.dram_tensor("xT", [P, SH], f16, kind="ExternalInput")
    idxA = nc.dram_tensor("idxA", [P, CA // 16], i16, kind="ExternalInput")
    idxB = nc.dram_tensor("idxB", [P, CB // 16], i16, kind="ExternalInput")
    dinv_in = nc.dram_tensor("dinv", [P, TILES], f32, kind="ExternalInput")
    w_in = nc.dram_tensor("w_in", [H, H], f16, kind="ExternalInput")
    b_in = nc.dram_tensor("b_in", [H, 1], f32, kind="ExternalInput")
    wl_in = nc.dram_tensor("wl", [H, NLAYERS * H], f16, kind="ExternalInput")
    wr_in = nc.dram_tensor("wr", [H, NLAYERS * H], f16, kind="ExternalInput")
    bl_in = nc.dram_tensor("bl", [H, NLAYERS], f32, kind="ExternalInput")
    wc1_in = nc.dram_tensor("wc1", [H, H], f16, kind="ExternalInput")
    bc1_in = nc.dram_tensor("bc1", [H, 1], f32, kind="ExternalInput")
    wc2_in = nc.dram_tensor("wc2", [H, ODIM], f16, kind="ExternalInput")
    bc2_in = nc.dram_tensor("bc2", [ODIM, 1], f32, kind="ExternalInput")
    ident_in = nc.dram_tensor("ident", [P, P], f16, kind="ExternalInput")
    dmask_in = nc.dram_tensor("dmask", [P, 1], f32, kind="ExternalInput")
    outT = nc.dram_tensor("outT", [ODIM, SH], f32, kind="ExternalOutput")

    with tile.TileContext(nc) as tc:
        with (
            tc.tile_pool(name="consts", bufs=1) as consts,
            tc.tile_pool(name="big", bufs=1) as big,
            tc.tile_pool(name="hT", bufs=2) as hTp,
            tc.tile_pool(name="aggTp", bufs=1) as aggTp,
            tc.tile_pool(name="hnm", bufs=1) as hnmp,
            tc.tile_pool(name="gA", bufs=GBUFS) as gAp,
            tc.tile_pool(name="gB", bufs=GBUFS) as gBp,
            tc.tile_pool(name="small", bufs=4) as small,
            tc.tile_pool(name="psagg", bufs=4, space="PSUM") as psagg,
            tc.tile_pool(name="pstp", bufs=2, space="PSUM") as pstp,
            tc.tile_pool(name="psz", bufs=1, space="PSUM") as psz,
            tc.tile_pool(name="dram", bufs=1, space="DRAM") as dram,
        ):
            # ---- load constants ----
            idxA_sb = consts.tile([P, CA // 16], i16)
            nc.sync.dma_start(out=idxA_sb[:], in_=idxA[:])
            idxB_sb = consts.tile([P, CB // 16], i16)
            nc.sync.dma_start(out=idxB_sb[:], in_=idxB[:])
            dinv_sb = consts.tile([P, TILES], f32)
            nc.sync.dma_start(out=dinv_sb[:], in_=dinv_in[:])
            w_in_sb = consts.tile([H, H], f16)
            nc.sync.dma_start(out=w_in_sb[:], in_=w_in[:])
            b_in_sb = consts.tile([H, 1], f32)
            nc.sync.dma_start(out=b_in_sb[:], in_=b_in[:])
            wl_sb = consts.tile([H, NLAYERS * H], f16)
            nc.sync.dma_start(out=wl_sb[:], in_=wl_in[:])
            wr_sb = consts.tile([H, NLAYERS * H], f16)
            nc.sync.dma_start(out=wr_sb[:], in_=wr_in[:])
            bl_sb = consts.tile([H, NLAYERS], f32)
            nc.sync.dma_start(out=bl_sb[:], in_=bl_in[:])
            wc1_sb = consts.tile([H, H], f16)
            nc.sync.dma_start(out=wc1_sb[:], in_=wc1_in[:])
            bc1_sb = consts.tile([H, 1], f32)
            nc.sync.dma_start(out=bc1_sb[:], in_=bc1_in[:])
            wc2_sb = consts.tile([H, ODIM], f16)
            nc.sync.dma_start(out=wc2_sb[:], in_=wc2_in[:])
            bc2_sb = consts.tile([ODIM, 1], f32)
            nc.sync.dma_start(out=bc2_sb[:], in_=bc2_in[:])
            ident_sb = consts.tile([P, P], f16)
            nc.sync.dma_start(out=ident_sb[:], in_=ident_in[:])
            dmask_sb = consts.tile([P, 1], f32)
            nc.sync.dma_start(out=dmask_sb[:], in_=dmask_in[:])
            xsum = big.tile([P, SH], f16)
            nc.vector.memset(xsum[:], 0.0)

            tabs = [dram.tile([NPAD, H], f16, addr_space="Shared",
                               name=f"tab{i}") for i in range(NLAYERS)]
            ag_ins = [dram.tile([SH, H], f16, name=f"ag_in{i}")
                      for i in range(NLAYERS)]

            swsems = tc.sems.swdge_block()

            # static per-layer gather job list: uniform GCHUNK-block
            # chunks over each window's flat block stream (chunks cross
            # tile boundaries; a 128-slot block always belongs to one tile)
            import bisect
            BA, BB = CA // P, CB // P
            tbA = [int(b) // P for b in baseA]  # tile boundaries in blocks
            tbB = [int(b) // P for b in baseB]
            JOBS = []  # (which, b0, s)
            for which, nb in ((0, BA), (1, BB)):
                for b0 in range(0, nb, GCHUNK):
                    JOBS.append((which, b0, min(GCHUNK, nb - b0)))

            def job_tile(j):
                which, b0, s = JOBS[j]
                tb = tbA if which == 0 else tbB
                return bisect.bisect_right(tb, b0) - 1

            order = sorted(range(len(JOBS)),
                           key=lambda j: (job_tile(j), JOBS[j][0],
                                          JOBS[j][1]))
            JOBS = [JOBS[j] for j in order]
            JOB_OF = {}  # (which, chunk_index) -> job position
            for pos, (which, b0, s) in enumerate(JOBS):
                JOB_OF[(which, b0 // GCHUNK)] = pos
            QOF = [j % NQ for j in range(len(JOBS))]  # strict RR
            NPREF = int(os.environ.get("GNN_NPREF", "8"))

            def emit_job(j, tab, gtiles, prep):
                which, b0, s = JOBS[j]
                pool, tag = (gAp, "gA") if which == 0 else (gBp, "gB")
                idx_sb = idxA_sb if which == 0 else idxB_sb
                tab_ap = tab[:] if which == 0 else tab[B0:, :]
                g = pool.tile([P, s * P], f16, tag=tag, name=tag)
                col0 = b0 * 8
                kw = {}
                if prep:
                    kw = dict(prepare_only=True,
                              sem=swsems[2 * QOF[j] + 1])
                nc.gpsimd.dma_gather(
                    g[:].rearrange("p (b e) -> p b e", e=H),
                    tab_ap,
                    idx_sb[:, col0 : col0 + s * 8],
                    s * P, s * P, H,
                    single_packet=False, queue_num=QOF[j], **kw)
                gtiles[j] = g

            def emit_prefix(tab, gtiles):
                """pre-generate descriptors for a prefix of the job list; the
                DMAs fire via trigger_dma once the AllGather lands. Capped at
                GBUFS jobs per pool so no prep reuses an untriggered prep's
                ring slot (that WAR would deadlock the engine)."""
                per_pool = {0: 0, 1: 0}
                done = 0
                for j in range(min(2 * NPREF, len(JOBS))):
                    which = JOBS[j][0]
                    if per_pool[which] >= GBUFS:
                        continue
                    per_pool[which] += 1
                    emit_job(j, tab, gtiles, prep=True)
                    done += 1
                    if done >= NPREF:
                        break

            def fire_prefix_triggers(gtiles, tab):
                """fire the prefix preps once tab's AllGather landed.

                Tile does not thread the collective->table dependency onto
                trigger_dma itself, so emit a tiny Pool-engine read of tab
                first: Tile gives IT the collective wait, and the engine's
                in-order dispatch holds the triggers behind it."""
                if not gtiles:
                    return
                probe = small.tile([1, H], f16, tag="tabprobe",
                                   name="tabprobe")
                nc.gpsimd.dma_start(out=probe[:], in_=tab[0:1, :])
                qs = {QOF[j] for j in gtiles}
                for q in sorted(qs):
                    nc.gpsimd.trigger_dma(count=None, queue_num=q)

            def gather_tile(t, tab, aggT, gtiles):
                """segment-sum one 128-dst tile into aggT cols; issues the
                tile's gathers unless they were pre-prepped."""
                da, db = int(DA[t]), int(DB[t])
                ps = psagg.tile([P, P], f32, tag="agg", name="psa")
                nblk = da + db
                k = 0
                for which, tb in ((0, tbA), (1, tbB)):
                    for b in range(tb[t], tb[t + 1]):
                        j = JOB_OF[(which, b // GCHUNK)]
                        if j not in gtiles:
                            emit_job(j, tab, gtiles, prep=False)
                        g = gtiles[j]
                        off = b - JOBS[j][1]
                        nc.tensor.matmul(
                            out=ps[:], lhsT=ident_sb[:],
                            rhs=g[:, off * P : (off + 1) * P],
                            start=(k == 0), stop=(k == nblk - 1))
                        k += 1
                # deg_inv scale + cast to fp16 on the idle DVE engine so the
                # ACT queue never gates the psagg release chain
                agg_nm = small.tile([P, P], f16, tag="aggnm", name="aggnm")
                nc.vector.tensor_scalar_mul(
                    out=agg_nm[:], in0=ps[:], scalar1=dinv_sb[:, t : t + 1])
                pst = pstp.tile([P, P], f16, tag="tp", name="pst2")
                nc.tensor.transpose(out=pst[:], in_=agg_nm[:],
                                    identity=ident_sb[:])
                nc.vector.tensor_scalar_mul(
                    out=aggT[:, t * P : (t + 1) * P], in0=pst[:], scalar1=1.0)

            def pack_tiles(hT_src, hnm, t0, t1):
                """transpose tiles [t0,t1) of hT_src (fp16) into hnm."""
                for t in range(t0, t1):
                    pst = pstp.tile([P, P], f16, tag="tp", name="pst")
                    nc.tensor.transpose(
                        out=pst[:], in_=hT_src[:, t * P : (t + 1) * P],
                        identity=ident_sb[:])
                    if t == TILES - 1:
                        # zero the dummy-node rows (tail partitions) via mask
                        nc.scalar.activation(
                            out=hnm[:, t * P : (t + 1) * P], in_=pst[:],
                            func=AF.Copy, scale=dmask_sb[:, 0:1])
                    else:
                        nc.scalar.activation(
                            out=hnm[:, t * P : (t + 1) * P], in_=pst[:],
                            func=AF.Copy)

            def fire_allgather(hnm, tab, ag_in):
                nc.sync.dma_start(
                    out=ag_in[:].rearrange("(t p) f -> p t f", p=P),
                    in_=hnm[:].rearrange("p (t f) -> p t f", f=P))
                if not os.environ.get("GNN_SKIP_AG"):
                    nc.gpsimd.collective_compute(
                        "AllGather", mybir.AluOpType.bypass,
                        replica_groups=[list(range(NC))],
                        ins=[ag_in[:]], outs=[tab[:]])

            # tile-groups matching the 512-col dense blocks; a small group
            # first primes the gather->matmul pipeline after each AllGather
            GROUPS = [(off // P, w // P, off, w) for off, w in DENSE_BLOCKS]
            GORDER = list(range(len(GROUPS)))

            # ---- layer 0: h0 = x @ W_in + b_in (pack interleaved) ----
            hT = hTp.tile([P, SH], f16, tag="hT", name="hT0")
            hnm = hnmp.tile([P, TILES * P], f16, tag="hnm", name="hnm0")
            for ci in range(0, len(GROUPS), 3):
                grp = GROUPS[ci : ci + 3]
                c0 = grp[0][2]
                cw = sum(g[3] for g in grp)
                xblk = small.tile([P, 3 * 512], f16, tag="xblk", name="xblk",
                                  bufs=2)
                nc.sync.dma_start(out=xblk[:, :cw], in_=xT[:, c0 : c0 + cw])
                for toff, tw, off, w in grp:
                    ps = psz.tile([P, 512], f32, tag="z", name="ps0")
                    nc.tensor.matmul(out=ps[:, :w], lhsT=w_in_sb[:],
                                     rhs=xblk[:, off - c0 : off - c0 + w],
                                     start=True, stop=True)
                    nc.scalar.activation(out=hT[:, off : off + w],
                                         in_=ps[:, :w],
                                         func=AF.Identity,
                                         bias=b_in_sb[:, 0:1])
                    pack_tiles(hT, hnm, toff, toff + tw)
            gtiles = {}
            emit_prefix(tabs[0], gtiles)
            fire_allgather(hnm, tabs[0], ag_ins[0])
            fire_prefix_triggers(gtiles, tabs[0])

            # ---- GNN layers: gather/dense/pack/classifier interleaved ----
            outT_sb = big.tile([ODIM, SH], f32)
            for layer in range(NLAYERS):
                last = layer == NLAYERS - 1
                aggT = aggTp.tile([P, SH], f16, tag="aggT", name="aggT")
                hT2 = hTp.tile([P, SH], f16, tag="hT", name=f"hT{layer + 1}")
                if not last:
                    hnm = hnmp.tile([P, TILES * P], f16, tag="hnm",
                                    name=f"hnm{layer + 1}")
                for gi in GORDER:
                    toff, tw, off, w = GROUPS[gi]
                    for t in range(toff, toff + tw):
                        gather_tile(t, tabs[layer], aggT, gtiles)
                    ps = psz.tile([P, 512], f32, tag="z", name=f"psz{layer}")
                    nc.tensor.matmul(
                        out=ps[:, :w], lhsT=wl_sb[:, layer * H : (layer + 1) * H],
                        rhs=hT[:, off : off + w], start=True, stop=False)
                    nc.tensor.matmul(
                        out=ps[:, :w], lhsT=wr_sb[:, layer * H : (layer + 1) * H],
                        rhs=aggT[:, off : off + w], start=False, stop=True,
                        skip_group_check=True)
                    nc.scalar.activation(
                        out=hT2[:, off : off + w], in_=ps[:, :w],
                        func=AF.Relu, bias=bl_sb[:, layer : layer + 1])
                    nc.vector.tensor_add(
                        out=xsum[:, off : off + w], in0=xsum[:, off : off + w],
                        in1=hT2[:, off : off + w])
                    if not last:
                        pack_tiles(hT2, hnm, toff, toff + tw)
                    else:
                        # classifier on the finished xsum columns
                        psc = psz.tile([P, 512], f32, tag="z", name="psc1")
                        nc.tensor.matmul(out=psc[:, :w], lhsT=wc1_sb[:],
                                         rhs=xsum[:, off : off + w],
                                         start=True, stop=True)
                        hc = small.tile([P, 512], f16, tag="hc", name="hc",
                                        bufs=2)
                        nc.scalar.activation(out=hc[:, :w], in_=psc[:, :w],
                                             func=AF.Relu, bias=bc1_sb[:, 0:1])
                        ps2 = psz.tile([ODIM, 512], f32, tag="z2", name="psc2",
                                       bufs=1)
                        nc.tensor.matmul(out=ps2[:, :w], lhsT=wc2_sb[:],
                                         rhs=hc[:, :w], start=True, stop=True)
                        nc.scalar.activation(out=outT_sb[:, off : off + w],
                                             in_=ps2[:, :w], func=AF.Identity,
                                             bias=bc2_sb[:, 0:1])
                if not last:
                    gtiles = {}
                    emit_prefix(tabs[layer + 1], gtiles)
                    fire_allgather(hnm, tabs[layer + 1], ag_ins[layer + 1])
                    fire_prefix_triggers(gtiles, tabs[layer + 1])
                hT = hT2
            nc.sync.dma_start(out=outT[:], in_=outT_sb[:])

    nc.compile()
    return nc


def kernel(x, edge_index, W_in, b_in, Wl, Wr, bl, Wc1, bc1, Wc2, bc2):
    global last_results
    from concourse.bass_utils import run_bass_kernel_spmd

    x = np.asarray(x, np.float32)
    edge_index = np.asarray(edge_index)
    meta = _prep(edge_index)
    DA, DB = meta["DA"], meta["DB"]
    baseA, baseB = meta["baseA"], meta["baseB"]
    CA, CB = int(baseA[-1]), int(baseB[-1])
    newid = meta["newid"]

    if NQ > 1:
        undo = _patch_swdge_lane_by_queue()
        try:
            nc = _build_program(DA, DB, baseA, baseB, CA, CB)
        finally:
            undo()
    else:
        nc = _build_program(DA, DB, baseA, baseB, CA, CB)

    # ---- per-core inputs ----
    dinv_full = np.ones(NC * SH, np.float32)
    dinv_full[newid] = meta["dinv"]
    x_full = np.zeros((NC * SH, H), np.float32)
    x_full[newid] = x

    ident = np.eye(P, dtype=np.float16)
    dmask = np.ones((P, 1), np.float32)
    dmask[P - N_DUMMY :] = 0.0
    wl_cat = np.concatenate([Wl[i] for i in range(NLAYERS)], 1).astype(np.float16)
    wr_cat = np.concatenate([Wr[i] for i in range(NLAYERS)], 1).astype(np.float16)
    bl_T = np.asarray(bl, np.float32).T.copy()  # [H, 3]

    in_maps = []
    for c in range(NC):
        sl = slice(c * SH, (c + 1) * SH)
        in_maps.append({
            "xT": x_full[sl].T.astype(np.float16).copy(),
            "idxA": np.tile(meta["arrA"][c], (8, 1)).astype(np.int16),
            "idxB": np.tile(meta["arrB"][c], (8, 1)).astype(np.int16),
            "dinv": dinv_full[sl].reshape(TILES, P).T.copy(),
            "w_in": np.asarray(W_in, np.float16),
            "b_in": np.asarray(b_in, np.float32).reshape(H, 1),
            "wl": wl_cat, "wr": wr_cat, "bl": bl_T,
            "wc1": np.asarray(Wc1, np.float16),
            "bc1": np.asarray(bc1, np.float32).reshape(H, 1),
            "wc2": np.asarray(Wc2, np.float16),
            "bc2": np.asarray(bc2, np.float32).reshape(ODIM, 1),
            "ident": ident,
            "dmask": dmask,
        })

    trace = bool(int(os.environ.get("GNN_TRACE", "0")))
    res = run_bass_kernel_spmd(nc, in_maps, list(range(NC)), trace=trace)
    last_results = res

    big = np.concatenate([res.results[c]["outT"] for c in range(NC)], axis=1)
    out = big.T[newid]  # [N, ODIM]
    return out.astype(np.float32)



# revision 20
# speedup vs baseline: 1.1386x; 1.0280x over previous
"""GraphSAGE-mean GNN (3 layers + classifier) on 8 Trainium2 NeuronCores.

Strategy (data-parallel over nodes, sharded by dst):
  - Nodes padded 50000 -> 50176, degree-sorted and dealt round-robin over the
    8 cores so every core sees a near-identical degree profile (SPMD: one
    compiled program for all cores). Per core: 6272 nodes = 49 tiles of 128.
  - Neighbor gather uses the custom Q7 dma_gather instruction (int16 indices,
    4 SWDGE queues). Since indices are signed int16 (< 32768), the gather
    table is addressed through two windows: A = rows [0, 32767] (sources on
    cores 0-4) and B = rows [17409, 50175] (sources on cores 5-7). Zero-slot
    padding points at dummy-node rows which are explicitly zeroed.
  - Gathered neighbor blocks [128 nodes x 128 feat] are accumulated on the
    tensor engine (identity matmuls into PSUM) to keep the vector engine off
    the SBUF port that SWDGE descriptor generation needs.
  - deg_inv scaling + PSUM->SBUF copies run on the scalar (ACT) engine.
  - Dense layer matmuls run feature-major: out = lhsT.T @ rhs with the weight
    as lhsT and h^T as rhs. Aggregates are transposed per-tile via PE.
  - Node features for the next layer's gather are re-transposed to node-major
    fp16, DMA'd to DRAM and AllGathered across the 8 cores.
"""

import os
import numpy as np

P = 128
NC = 8
N = 50000
E = 800000
NPAD = 50176
SH = 6272  # nodes per core
TILES = 49
H = 128
NLAYERS = 3
ODIM = 40
ACUT = 31360  # sources with new id < ACUT use window A (cores 0-4)
B0 = 17409  # window B base row; idx = id - B0 (<= 32766 for id < 50176)
AZERO = ACUT - 1  # core 4's last node: a dummy (zeroed) row, < 32768
BZERO = NPAD - 1 - B0  # core 7's last node as B-window index
NQ = int(os.environ.get('GNN_NQ', '4'))
GCHUNK = int(os.environ.get('GNN_GCHUNK', '16'))  # 128-blocks per gather call
GBUFS = int(os.environ.get('GNN_GBUFS', '5'))
DENSE_BLOCKS = [(i * 512, 512) for i in range(12)] + [(6144, 128)]
N_DUMMY = SH - N // NC  # 22 dummy nodes per core, at the tail of tile 48

last_results = None


def _prep(edge_index):
    """Host-side graph preprocessing: permutation, per-tile neighbor slots."""
    src = edge_index[0].astype(np.int64)
    dst = edge_index[1].astype(np.int64)
    deg = np.bincount(dst, minlength=N).astype(np.float32)
    dinv = 1.0 / np.maximum(deg, 1.0)

    # pass 1: global degree sort, deal round-robin -> core assignment
    order = np.argsort(-deg, kind="stable")  # rank -> orig node
    core_of = np.empty(N, np.int64)
    core_of[order] = np.arange(N) % NC

    # pass 2: within-core order by (degA, degB) desc; degA counts in-edges
    # whose source core is in 0..4 (window A). Two re-deal rounds align the
    # per-core (degA, degB) profiles, shrinking the cross-core tile maxima.
    for _ in range(2):
        srcA = core_of[src] < 5
        degA = np.bincount(dst[srcA], minlength=N).astype(np.int64)
        degB = np.bincount(dst[~srcA], minlength=N).astype(np.int64)
        key = (degA // 10) * 1000000 + (degB // 4) * 1000 \
            + degA % 10 * 10 + degB % 4
        order = np.argsort(-key, kind="stable")
        core_of = np.empty(N, np.int64)
        core_of[order] = np.arange(N) % NC
    srcA = core_of[src] < 5
    degA = np.bincount(dst[srcA], minlength=N).astype(np.int64)
    degB = np.bincount(dst[~srcA], minlength=N).astype(np.int64)

    newid = np.empty(N, np.int64)
    for c in range(NC):
        nodes = np.flatnonzero(core_of == c)
        key = (degA[nodes] // 10) * 1000000 + (degB[nodes] // 4) * 1000 \
            + degA[nodes] % 10 * 10 + degB[nodes] % 4
        nodes = nodes[np.argsort(-key, kind="stable")]
        newid[nodes] = c * SH + np.arange(len(nodes))  # dummies occupy the tail

    ns = newid[src]
    nd = newid[dst]
    isA = ns < ACUT

    # per-(core, tile) block counts; uniform across cores for SPMD
    def build(mask, zero_idx, rebase):
        ns_h, nd_h = ns[mask], nd[mask]
        o = np.argsort(nd_h, kind="stable")
        ns_h, nd_h = ns_h[o], nd_h[o]
        # cumcount within each dst node
        n_edges = len(nd_h)
        if n_edges == 0:
            return np.zeros(NC * NPAD // P // NC, np.int64), None, None
        firsts = np.r_[0, np.flatnonzero(np.diff(nd_h)) + 1]
        grp = np.zeros(n_edges, np.int64)
        grp[firsts] = 1
        grp = np.cumsum(grp) - 1
        d_in_node = np.arange(n_edges) - firsts[grp]
        cnt = np.bincount(nd_h, minlength=NC * SH)  # per new-node count
        cnt_t = cnt.reshape(NC, TILES, P)
        D_t = cnt_t.max(axis=2).max(axis=0)  # [TILES], max over cores
        base = np.r_[0, np.cumsum(D_t * P)]  # slot base per tile
        CA = int(base[-1])  # slots per core
        arr16 = np.full((NC, 16, CA // 16), zero_idx, np.int64)
        c_h = nd_h // SH
        pos = nd_h % SH
        t_h = pos // P
        p_h = pos % P
        flat = base[t_h] + d_in_node * P + p_h
        val = ns_h - rebase
        arr16[c_h, flat % 16, flat // 16] = val
        return D_t, base, arr16

    DA, baseA, arrA = build(isA, AZERO, 0)
    DB, baseB, arrB = build(~isA, BZERO, B0)

    inv = np.empty(N, np.int64)  # only defined for real nodes
    inv = newid[np.arange(N)]  # orig -> new
    return dict(
        dinv=dinv, newid=inv, DA=DA, DB=DB, baseA=baseA, baseB=baseB,
        arrA=arrA, arrB=arrB,
    )



def _patch_swdge_lane_by_queue():
    """Bind DMASW sem lanes to SWDGE queues so multi-queue SWDGE keeps the
    per-queue semaphore invariant the ucode reclaim needs. Lane 2q serves
    queue q's normal (gen_mode=0) DMAs; lane 2q+1 its PREPARE_ONLY preps —
    deterministic, so trace-time sem= choices match pass-1 lane assignment.
    Returns an undo function."""
    import concourse.tile_sem_assignment as tsa
    import concourse.mybir as mybir
    from concourse import bass_isa
    from concourse.tile_scheduler import DMAInst

    orig = tsa.TileClockTick._assign_tick

    def patched(self, inst):
        try:
            is_pool_dma = (
                isinstance(inst, DMAInst)
                and inst.engine == mybir.EngineType.Pool
                and not isinstance(inst, bass_isa.UserSyncedRemoteDMADescs)
            )
        except Exception:
            is_pool_dma = False
        if is_pool_dma:
            q = int(getattr(inst, "queue_num", 0) or 0)
            counters = getattr(self, "_queue_lane_ctr", None)
            if counters is None:
                counters = {}
                self._queue_lane_ctr = counters
            k = counters.get(q, 0)
            counters[q] = k + 1
            lanes_per_q = max(1, self.swdge_sem_count // NQ)
            self.next_sw_dma_idx = (q * lanes_per_q + k % lanes_per_q) % (
                self.swdge_sem_count
            )
        return orig(self, inst)

    tsa.TileClockTick._assign_tick = patched

    def undo():
        tsa.TileClockTick._assign_tick = orig

    return undo


def _build_program(DA, DB, baseA, baseB, CA, CB):
    import concourse.bass as bass
    import concourse.bacc as bacc
    import concourse.tile as tile
    import concourse.mybir as mybir

    f32 = mybir.dt.float32
    f16 = mybir.dt.float16
    i16 = mybir.dt.int16
    AF = mybir.ActivationFunctionType

    nc = bacc.Bacc("TRN2", target_bir_lowering=False, debug=False,
                   num_devices=NC, num_swdge_queues=NQ)

    # ---- I/O ----
    xT = nc.dram_tensor("xT", [P, SH], f16, kind="ExternalInput")
    idxA = nc.dram_tensor("idxA", [P, CA // 16], i16, kind="ExternalInput")
    idxB = nc.dram_tensor("idxB", [P, CB // 16], i16, kind="ExternalInput")
    dinv_in = nc.dram_tensor("dinv", [P, TILES], f32, kind="ExternalInput")
    w_in = nc.dram_tensor("w_in", [H, H], f16, kind="ExternalInput")
    b_in = nc.dram_tensor("b_in", [H, 1], f32, kind="ExternalInput")
    wl_in = nc.dram_tensor("wl", [H, NLAYERS * H], f16, kind="ExternalInput")
    wr_in = nc.dram_tensor("wr", [H, NLAYERS * H], f16, kind="ExternalInput")
    bl_in = nc.dram_tensor("bl", [H, NLAYERS], f32, kind="ExternalInput")
    wc1_in = nc.dram_tensor("wc1", [H, H], f16, kind="ExternalInput")
    bc1_in = nc.dram_tensor("bc1", [H, 1], f32, kind="ExternalInput")
    wc2_in = nc.dram_tensor("wc2", [H, ODIM], f16, kind="ExternalInput")
    bc2_in = nc.dram_tensor("bc2", [ODIM, 1], f32, kind="ExternalInput")
    ident_in = nc.dram_tensor("ident", [P, P], f16, kind="ExternalInput")
    dmask_in = nc.dram_tensor("dmask", [P, 1], f32, kind="ExternalInput")
    outT = nc.dram_tensor("outT", [ODIM, SH], f32, kind="ExternalOutput")

    with tile.TileContext(nc) as tc:
        with (
            tc.tile_pool(name="consts", bufs=1) as consts,
            tc.tile_pool(name="big", bufs=1) as big,
            tc.tile_pool(name="hT", bufs=2) as hTp,
            tc.tile_pool(name="aggTp", bufs=1) as aggTp,
            tc.tile_pool(name="hnm", bufs=1) as hnmp,
            tc.tile_pool(name="gA", bufs=GBUFS) as gAp,
            tc.tile_pool(name="gB", bufs=GBUFS) as gBp,
            tc.tile_pool(name="small", bufs=4) as small,
            tc.tile_pool(name="psagg", bufs=4, space="PSUM") as psagg,
            tc.tile_pool(name="pstp", bufs=2, space="PSUM") as pstp,
            tc.tile_pool(name="psz", bufs=1, space="PSUM") as psz,
            tc.tile_pool(name="dram", bufs=1, space="DRAM") as dram,
        ):
            # ---- load constants ----
            idxA_sb = consts.tile([P, CA // 16], i16)
            nc.sync.dma_start(out=idxA_sb[:], in_=idxA[:])
            idxB_sb = consts.tile([P, CB // 16], i16)
            nc.sync.dma_start(out=idxB_sb[:], in_=idxB[:])
            dinv_sb = consts.tile([P, TILES], f32)
            nc.sync.dma_start(out=dinv_sb[:], in_=dinv_in[:])
            w_in_sb = consts.tile([H, H], f16)
            nc.sync.dma_start(out=w_in_sb[:], in_=w_in[:])
            b_in_sb = consts.tile([H, 1], f32)
            nc.sync.dma_start(out=b_in_sb[:], in_=b_in[:])
            wl_sb = consts.tile([H, NLAYERS * H], f16)
            nc.sync.dma_start(out=wl_sb[:], in_=wl_in[:])
            wr_sb = consts.tile([H, NLAYERS * H], f16)
            nc.sync.dma_start(out=wr_sb[:], in_=wr_in[:])
            bl_sb = consts.tile([H, NLAYERS], f32)
            nc.sync.dma_start(out=bl_sb[:], in_=bl_in[:])
            wc1_sb = consts.tile([H, H], f16)
            nc.sync.dma_start(out=wc1_sb[:], in_=wc1_in[:])
            bc1_sb = consts.tile([H, 1], f32)
            nc.sync.dma_start(out=bc1_sb[:], in_=bc1_in[:])
            wc2_sb = consts.tile([H, ODIM], f16)
            nc.sync.dma_start(out=wc2_sb[:], in_=wc2_in[:])
            bc2_sb = consts.tile([ODIM, 1], f32)
            nc.sync.dma_start(out=bc2_sb[:], in_=bc2_in[:])
            ident_sb = consts.tile([P, P], f16)
            nc.sync.dma_start(out=ident_sb[:], in_=ident_in[:])
            dmask_sb = consts.tile([P, 1], f32)
            nc.sync.dma_start(out=dmask_sb[:], in_=dmask_in[:])
            xsum = big.tile([P, SH], f16)
            nc.vector.memset(xsum[:], 0.0)
            barsrc = consts.tile([2, 1], f32)
            nc.vector.memset(barsrc[:], 0.0)

            tabs = [dram.tile([NPAD, H], f16, addr_space="Shared",
                               name=f"tab{i}") for i in range(NLAYERS)]
            ag_ins = [dram.tile([SH, H], f16, name=f"ag_in{i}")
                      for i in range(NLAYERS)]

            swsems = tc.sems.swdge_block()

            # static per-layer gather job list: uniform GCHUNK-block
            # chunks over each window's flat block stream (chunks cross
            # tile boundaries; a 128-slot block always belongs to one tile)
            import bisect
            BA, BB = CA // P, CB // P
            tbA = [int(b) // P for b in baseA]  # tile boundaries in blocks
            tbB = [int(b) // P for b in baseB]
            JOBS = []  # (which, b0, s)
            for which, nb in ((0, BA), (1, BB)):
                for b0 in range(0, nb, GCHUNK):
                    JOBS.append((which, b0, min(GCHUNK, nb - b0)))

            def job_tile(j):
                which, b0, s = JOBS[j]
                tb = tbA if which == 0 else tbB
                return bisect.bisect_right(tb, b0) - 1

            order = sorted(range(len(JOBS)),
                           key=lambda j: (job_tile(j), JOBS[j][0],
                                          JOBS[j][1]))
            JOBS = [JOBS[j] for j in order]
            JOB_OF = {}  # (which, chunk_index) -> job position
            for pos, (which, b0, s) in enumerate(JOBS):
                JOB_OF[(which, b0 // GCHUNK)] = pos
            QOF = [j % NQ for j in range(len(JOBS))]  # strict RR
            NPREF = int(os.environ.get("GNN_NPREF", "8"))

            def emit_job(j, tab, gtiles, prep):
                which, b0, s = JOBS[j]
                pool, tag = (gAp, "gA") if which == 0 else (gBp, "gB")
                idx_sb = idxA_sb if which == 0 else idxB_sb
                tab_ap = tab[:] if which == 0 else tab[B0:, :]
                g = pool.tile([P, s * P], f16, tag=tag, name=tag)
                col0 = b0 * 8
                kw = {}
                if prep:
                    kw = dict(prepare_only=True,
                              sem=swsems[2 * QOF[j] + 1])
                nc.gpsimd.dma_gather(
                    g[:].rearrange("p (b e) -> p b e", e=H),
                    tab_ap,
                    idx_sb[:, col0 : col0 + s * 8],
                    s * P, s * P, H,
                    single_packet=False, queue_num=QOF[j], **kw)
                gtiles[j] = g

            def emit_prefix(tab, gtiles):
                """pre-generate descriptors for a prefix of the job list; the
                DMAs fire via trigger_dma once the AllGather lands. Capped at
                GBUFS jobs per pool so no prep reuses an untriggered prep's
                ring slot (that WAR would deadlock the engine)."""
                per_pool = {0: 0, 1: 0}
                done = 0
                for j in range(min(2 * NPREF, len(JOBS))):
                    which = JOBS[j][0]
                    if per_pool[which] >= GBUFS:
                        continue
                    per_pool[which] += 1
                    emit_job(j, tab, gtiles, prep=True)
                    done += 1
                    if done >= NPREF:
                        break

            def fire_prefix_triggers(gtiles, tab):
                """fire the prefix preps once tab's AllGather landed.

                Tile does not thread the collective->table dependency onto
                trigger_dma itself, so emit a tiny Pool-engine read of tab
                first: Tile gives IT the collective wait, and the engine's
                in-order dispatch holds the triggers behind it."""
                if not gtiles:
                    return
                probe = small.tile([1, H], f16, tag="tabprobe",
                                   name="tabprobe")
                nc.gpsimd.dma_start(out=probe[:], in_=tab[0:1, :])
                qs = {QOF[j] for j in gtiles}
                for q in sorted(qs):
                    nc.gpsimd.trigger_dma(count=None, queue_num=q)

            def gather_tile(t, tab, aggT, gtiles):
                """segment-sum one 128-dst tile into aggT cols; issues the
                tile's gathers unless they were pre-prepped."""
                da, db = int(DA[t]), int(DB[t])
                ps = psagg.tile([P, P], f32, tag="agg", name="psa")
                skipb = bool(int(os.environ.get("GNN_SKIPB", "0")))
                nblk = da if skipb else da + db
                k = 0
                streams = ((0, tbA),) if skipb else ((0, tbA), (1, tbB))
                for which, tb in streams:
                    for b in range(tb[t], tb[t + 1]):
                        j = JOB_OF[(which, b // GCHUNK)]
                        if j not in gtiles:
                            emit_job(j, tab, gtiles, prep=False)
                        g = gtiles[j]
                        off = b - JOBS[j][1]
                        nc.tensor.matmul(
                            out=ps[:], lhsT=ident_sb[:],
                            rhs=g[:, off * P : (off + 1) * P],
                            start=(k == 0), stop=(k == nblk - 1))
                        k += 1
                # deg_inv scale + cast to fp16 on the idle DVE engine so the
                # ACT queue never gates the psagg release chain
                agg_nm = small.tile([P, P], f16, tag="aggnm", name="aggnm")
                nc.vector.tensor_scalar_mul(
                    out=agg_nm[:], in0=ps[:], scalar1=dinv_sb[:, t : t + 1])
                pst = pstp.tile([P, P], f16, tag="tp", name="pst2")
                nc.tensor.transpose(out=pst[:], in_=agg_nm[:],
                                    identity=ident_sb[:])
                nc.vector.tensor_scalar_mul(
                    out=aggT[:, t * P : (t + 1) * P], in0=pst[:], scalar1=1.0)

            def pack_tiles(hT_src, hnm, t0, t1):
                """transpose tiles [t0,t1) of hT_src (fp16) into hnm."""
                for t in range(t0, t1):
                    pst = pstp.tile([P, P], f16, tag="tp", name="pst")
                    nc.tensor.transpose(
                        out=pst[:], in_=hT_src[:, t * P : (t + 1) * P],
                        identity=ident_sb[:])
                    if t == TILES - 1:
                        # zero the dummy-node rows (tail partitions) via mask
                        nc.scalar.activation(
                            out=hnm[:, t * P : (t + 1) * P], in_=pst[:],
                            func=AF.Copy, scale=dmask_sb[:, 0:1])
                    else:
                        nc.scalar.activation(
                            out=hnm[:, t * P : (t + 1) * P], in_=pst[:],
                            func=AF.Copy)

            barrier_bufs = [dram.tile([16, 1], f32, addr_space="Shared",
                                      name=f"bar{i}")
                            for i in range(NLAYERS + 1)]
            bar_in = dram.tile([2, 1], f32, name="bar_in")
            nc.sync.dma_start(out=bar_in[:], in_=barsrc[:])
            bar_ctr = [0]

            def fire_allgather(hnm, tab, ag_in):
                nc.sync.dma_start(
                    out=ag_in[:].rearrange("(t p) f -> p t f", p=P),
                    in_=hnm[:].rearrange("p (t f) -> p t f", f=P))
                if not os.environ.get("GNN_SKIP_AG"):
                    nc.gpsimd.collective_compute(
                        "AllGather", mybir.AluOpType.bypass,
                        replica_groups=[list(range(NC))],
                        ins=[ag_in[:]], outs=[tab[:]])
                    if int(os.environ.get("GNN_BARRIER", "1")):
                        # tiny post-AG AllGather as a cross-core barrier so
                        # every core starts the next gather phase together
                        bar = barrier_bufs[bar_ctr[0]]
                        bar_ctr[0] += 1
                        nc.gpsimd.collective_compute(
                            "AllGather", mybir.AluOpType.bypass,
                            replica_groups=[list(range(NC))],
                            ins=[bar_in[:]], outs=[bar[:]])
                        probe2 = small.tile([1, 1], f32, tag="barprobe",
                                            name="barprobe")
                        nc.gpsimd.dma_start(out=probe2[:], in_=bar[0:1, :])

            # tile-groups matching the 512-col dense blocks; a small group
            # first primes the gather->matmul pipeline after each AllGather
            GROUPS = [(off // P, w // P, off, w) for off, w in DENSE_BLOCKS]
            GORDER = list(range(len(GROUPS)))

            # ---- layer 0: h0 = x @ W_in + b_in (pack interleaved) ----
            hT = hTp.tile([P, SH], f16, tag="hT", name="hT0")
            hnm = hnmp.tile([P, TILES * P], f16, tag="hnm", name="hnm0")
            for ci in range(0, len(GROUPS), 3):
                grp = GROUPS[ci : ci + 3]
                c0 = grp[0][2]
                cw = sum(g[3] for g in grp)
                xblk = small.tile([P, 3 * 512], f16, tag="xblk", name="xblk",
                                  bufs=2)
                nc.sync.dma_start(out=xblk[:, :cw], in_=xT[:, c0 : c0 + cw])
                for toff, tw, off, w in grp:
                    ps = psz.tile([P, 512], f32, tag="z", name="ps0")
                    nc.tensor.matmul(out=ps[:, :w], lhsT=w_in_sb[:],
                                     rhs=xblk[:, off - c0 : off - c0 + w],
                                     start=True, stop=True)
                    nc.scalar.activation(out=hT[:, off : off + w],
                                         in_=ps[:, :w],
                                         func=AF.Identity,
                                         bias=b_in_sb[:, 0:1])
                    pack_tiles(hT, hnm, toff, toff + tw)
            gtiles = {}
            emit_prefix(tabs[0], gtiles)
            fire_allgather(hnm, tabs[0], ag_ins[0])
            fire_prefix_triggers(gtiles, tabs[0])

            # ---- GNN layers: gather/dense/pack/classifier interleaved ----
            outT_sb = big.tile([ODIM, SH], f32)
            for layer in range(NLAYERS):
                last = layer == NLAYERS - 1
                aggT = aggTp.tile([P, SH], f16, tag="aggT", name="aggT")
                hT2 = hTp.tile([P, SH], f16, tag="hT", name=f"hT{layer + 1}")
                if not last:
                    hnm = hnmp.tile([P, TILES * P], f16, tag="hnm",
                                    name=f"hnm{layer + 1}")
                for gi in GORDER:
                    toff, tw, off, w = GROUPS[gi]
                    for t in range(toff, toff + tw):
                        gather_tile(t, tabs[layer], aggT, gtiles)
                    ps = psz.tile([P, 512], f32, tag="z", name=f"psz{layer}")
                    nc.tensor.matmul(
                        out=ps[:, :w], lhsT=wl_sb[:, layer * H : (layer + 1) * H],
                        rhs=hT[:, off : off + w], start=True, stop=False)
                    nc.tensor.matmul(
                        out=ps[:, :w], lhsT=wr_sb[:, layer * H : (layer + 1) * H],
                        rhs=aggT[:, off : off + w], start=False, stop=True,
                        skip_group_check=True)
                    nc.scalar.activation(
                        out=hT2[:, off : off + w], in_=ps[:, :w],
                        func=AF.Relu, bias=bl_sb[:, layer : layer + 1])
                    nc.vector.tensor_add(
                        out=xsum[:, off : off + w], in0=xsum[:, off : off + w],
                        in1=hT2[:, off : off + w])
                    if not last:
                        pack_tiles(hT2, hnm, toff, toff + tw)
                    else:
                        # classifier on the finished xsum columns
                        psc = psz.tile([P, 512], f32, tag="z", name="psc1")
                        nc.tensor.matmul(out=psc[:, :w], lhsT=wc1_sb[:],
                                         rhs=xsum[:, off : off + w],
                                         start=True, stop=True)
                        hc = small.tile([P, 512], f16, tag="hc", name="hc",
                                        bufs=2)
                        nc.scalar.activation(out=hc[:, :w], in_=psc[:, :w],
                                             func=AF.Relu, bias=bc1_sb[:, 0:1])
                        ps2 = psz.tile([ODIM, 512], f32, tag="z2", name="psc2",
                                       bufs=1)
                        nc.tensor.matmul(out=ps2[:, :w], lhsT=wc2_sb[:],
                                         rhs=hc[:, :w], start=True, stop=True)
                        nc.scalar.activation(out=outT_sb[:, off : off + w],
                                             in_=ps2[:, :w], func=AF.Identity,
                                             bias=bc2_sb[:, 0:1])
                if not last:
                    gtiles = {}
                    emit_prefix(tabs[layer + 1], gtiles)
                    fire_allgather(hnm, tabs[layer + 1], ag_ins[layer + 1])
                    fire_prefix_triggers(gtiles, tabs[layer + 1])
                hT = hT2
            nc.sync.dma_start(out=outT[:], in_=outT_sb[:])

    nc.compile()
    return nc


def kernel(x, edge_index, W_in, b_in, Wl, Wr, bl, Wc1, bc1, Wc2, bc2):
    global last_results
    from concourse.bass_utils import run_bass_kernel_spmd

    x = np.asarray(x, np.float32)
    edge_index = np.asarray(edge_index)
    meta = _prep(edge_index)
    DA, DB = meta["DA"], meta["DB"]
    baseA, baseB = meta["baseA"], meta["baseB"]
    CA, CB = int(baseA[-1]), int(baseB[-1])
    newid = meta["newid"]

    if NQ > 1:
        undo = _patch_swdge_lane_by_queue()
        try:
            nc = _build_program(DA, DB, baseA, baseB, CA, CB)
        finally:
            undo()
    else:
        nc = _build_program(DA, DB, baseA, baseB, CA, CB)

    # ---- per-core inputs ----
    dinv_full = np.ones(NC * SH, np.float32)
    dinv_full[newid] = meta["dinv"]
    x_full = np.zeros((NC * SH, H), np.float32)
    x_full[newid] = x

    ident = np.eye(P, dtype=np.float16)
    dmask = np.ones((P, 1), np.float32)
    dmask[P - N_DUMMY :] = 0.0
    wl_cat = np.concatenate([Wl[i] for i in range(NLAYERS)], 1).astype(np.float16)
    wr_cat = np.concatenate([Wr[i] for i in range(NLAYERS)], 1).astype(np.float16)
    bl_T = np.asarray(bl, np.float32).T.copy()  # [H, 3]

    in_maps = []
    for c in range(NC):
        sl = slice(c * SH, (c + 1) * SH)
        in_maps.append({
            "xT": x_full[sl].T.astype(np.float16).copy(),
            "idxA": np.tile(meta["arrA"][c], (8, 1)).astype(np.int16),
            "idxB": np.tile(meta["arrB"][c], (8, 1)).astype(np.int16),
            "dinv": dinv_full[sl].reshape(TILES, P).T.copy(),
            "w_in": np.asarray(W_in, np.float16),
            "b_in": np.asarray(b_in, np.float32).reshape(H, 1),
            "wl": wl_cat, "wr": wr_cat, "bl": bl_T,
            "wc1": np.asarray(Wc1, np.float16),
            "bc1": np.asarray(bc1, np.float32).reshape(H, 1),
            "wc2": np.asarray(Wc2, np.float16),
            "bc2": np.asarray(bc2, np.float32).reshape(ODIM, 1),
            "ident": ident,
            "dmask": dmask,
        })

    trace = bool(int(os.environ.get("GNN_TRACE", "0")))
    res = run_bass_kernel_spmd(nc, in_maps, list(range(NC)), trace=trace)
    last_results = res

    big = np.concatenate([res.results[c]["outT"] for c in range(NC)], axis=1)
    out = big.T[newid]  # [N, ODIM]
    return out.astype(np.float32)

